# revision 1
# baseline (speedup 1.0000x reference)
"""Causal self-attention (B=8, T=1024, C=768, H=12, Dh=64) on 8 TRN2 NeuronCores.

Sharding: batch data-parallel. Core b computes the full attention block for
batch element b (weights replicated). No collectives.

Per-core dataflow (fp32 data; matmuls run as float32r):
  1. x [T,C] -> xT [C,T] via PE transposes.
  2. Q^T,K^T [C,T] = W^T @ xT (W_qkv Q/K columns streamed per head-pair);
     V [T, C] = x @ W_v computed directly in [t-part, c-free] layout, stored
     per head with an extra all-ones column (V_aug [k, 65]) so the P@V matmul
     also accumulates softmax denominators.
  3. Per head h: for each k-block kb (128 wide), S^T tile [k=128, q] over the
     causal-valid q range only; P^T = exp(S^T/8) on ACT (scores are ~N(0,1),
     so no max-subtraction is needed); causal mask applied by affine_select
     filling 0 after exp; O'^T [65, q] += V_aug^T @ P^T.  Row 64 of O' is the
     softmax denominator: reciprocal -> partition_broadcast -> multiply
     normalizes O^T, written into OT [C, T] (partition = channel).
  4. y [T,C] = OT-as-lhsT @ W_out + b_out, DMA to DRAM.
"""

import numpy as np

B, T, C = 8, 1024, 768
H, D = 12, 64
TB = T // 128  # 8 t/k blocks
CB = C // 128  # 6 channel blocks
NCORES = 8

_CACHE = {}


def _ensure_path():
    import sys

    for p in ("/opt/trn_rl_repo",):
        if p not in sys.path:
            sys.path.insert(0, p)


def _emit(nc, tc, tile, mybir, make_identity):
    f32 = mybir.dt.float32
    f32r = mybir.dt.float32r
    Exp = mybir.ActivationFunctionType.Exp
    Ln = mybir.ActivationFunctionType.Ln
    isge = mybir.AluOpType.is_ge

    x_d = nc.dram_tensor("x", [T, C], f32, kind="ExternalInput")
    wqkv_d = nc.dram_tensor("W_qkv", [C, 3 * C], f32r, kind="ExternalInput")
    bqkv_d = nc.dram_tensor("b_qkv", [3 * C], f32, kind="ExternalInput")
    wout_d = nc.dram_tensor("W_out", [C, C], f32r, kind="ExternalInput")
    bout_d = nc.dram_tensor("b_out", [C], f32, kind="ExternalInput")
    y_d = nc.dram_tensor("y_out", [T, C], f32, kind="ExternalOutput")

    with (
        tc.tile_pool(name="const", bufs=1) as const_pool,
        tc.tile_pool(name="wres", bufs=1) as wres,
        tc.tile_pool(name="wqkp", bufs=2) as wqk_pool,
        tc.tile_pool(name="xin", bufs=4) as xin_pool,
        tc.tile_pool(name="big", bufs=1) as big,
        tc.tile_pool(name="qktp", bufs=2) as qkt_pool,
        tc.tile_pool(name="ptp", bufs=4) as pt_pool,
        tc.tile_pool(name="yp", bufs=2) as y_pool,
        tc.tile_pool(name="smallp", bufs=1) as small_pool,
        tc.tile_pool(name="mmp", bufs=2, space="PSUM") as mm_psum,
        tc.tile_pool(name="stp", bufs=2, space="PSUM") as st_psum,
        tc.tile_pool(name="op", bufs=1, space="PSUM") as o_psum,
    ):
        # ---------- constants ----------
        ident = const_pool.tile([128, 128], f32, name="ident")
        make_identity(nc, ident[:])


        xT = big.tile([128, CB, T], f32r, name="xT")
        V = big.tile([128, TB, H, D + 1], f32r, name="V")
        OT = [big.tile([128, T], f32r, name=f"OT{cb}", tag=f"OT{cb}") for cb in range(CB)]

        # ---------- load + transpose x ----------
        for tb in range(TB):
            x_in = xin_pool.tile([128, C], f32, name="x_in", tag="x_in")
            nc.sync.dma_start(x_in[:], x_d[tb * 128 : (tb + 1) * 128, :])
            ps_a = mm_psum.tile([128, 512], f32, name="ps_a", tag="mm")
            for i in range(4):
                nc.tensor.transpose(
                    ps_a[:, i * 128 : (i + 1) * 128],
                    x_in[:, i * 128 : (i + 1) * 128],
                    ident[:],
                )
            nc.vector.tensor_copy(
                xT[:, 0:4, tb * 128 : (tb + 1) * 128],
                ps_a[:].rearrange("p (c t) -> p c t", c=4),
            )
            ps_b = mm_psum.tile([128, 512], f32, name="ps_b", tag="mm")
            for i in range(2):
                cb = 4 + i
                nc.tensor.transpose(
                    ps_b[:, i * 128 : (i + 1) * 128],
                    x_in[:, cb * 128 : (cb + 1) * 128],
                    ident[:],
                )
            nc.vector.tensor_copy(
                xT[:, 4:6, tb * 128 : (tb + 1) * 128],
                ps_b[:, 0:256].rearrange("p (c t) -> p c t", c=2),
            )


        # causal masks: maskd[kp, qf] = 1 if qf >= kp (diagonal block);
        # maskw[kp, qf] = 1 if qf >= kp + 128 (junk cols + diagonal, 256 wide)
        maskd = const_pool.tile([128, 128], f32, name="maskd")
        nc.gpsimd.memset(maskd[:], 1.0)
        nc.gpsimd.affine_select(
            out=maskd[:], in_=maskd[:], compare_op=isge, fill=0.0,
            base=0, channel_multiplier=-1, pattern=[[1, 128]],
        )
        maskw = const_pool.tile([128, 256], f32, name="maskw")
        nc.gpsimd.memset(maskw[:], 1.0)
        nc.gpsimd.affine_select(
            out=maskw[:], in_=maskw[:], compare_op=isge, fill=0.0,
            base=-128, channel_multiplier=-1, pattern=[[1, 256]],
        )
        nc.gpsimd.memset(V[:, :, :, D : D + 1].bitcast(f32), 1.0)

        # b_qkv as [128, 18]: column m holds channels m*128..m*128+127
        bqk = const_pool.tile([128, 18], f32, name="bqk")
        nc.scalar.dma_start(bqk[:], bqkv_d[:].rearrange("(m p) -> p m", p=128))

        bv_bc = const_pool.tile([128, C], f32, name="bv_bc")
        nc.scalar.dma_start(bv_bc[0:1, :], bqkv_d[2 * C : 3 * C][None, :])
        nc.gpsimd.partition_broadcast(bv_bc[:], bv_bc[0:1, :])

        bo_bc = const_pool.tile([128, C], f32, name="bo_bc")
        nc.scalar.dma_start(bo_bc[0:1, :], bout_d[:][None, :])
        nc.gpsimd.partition_broadcast(bo_bc[:], bo_bc[0:1, :])

        wv = wres.tile([128, CB, C], f32r, name="wv")
        wout = wres.tile([128, CB, C], f32r, name="wout")
        for cb in range(CB):
            nc.scalar.dma_start(
                wv[:, cb, :], wqkv_d[cb * 128 : (cb + 1) * 128, 2 * C : 3 * C]
            )
        for cb in range(CB):
            nc.scalar.dma_start(wout[:, cb, :], wout_d[cb * 128 : (cb + 1) * 128, :])

        # ---------- V projection: V[t, c] = x @ W_v + b_v ----------
        for tb in range(TB):
            for ch in range(2):  # two 384-wide channel chunks
                ps = mm_psum.tile([128, 512], f32, name="ps_v", tag="mm")
                for cb in range(CB):
                    nc.tensor.matmul(
                        ps[:, 0:384],
                        xT[:, cb, tb * 128 : (tb + 1) * 128],
                        wv[:, cb, ch * 384 : (ch + 1) * 384],
                        start=(cb == 0),
                        stop=(cb == CB - 1),
                    )
                nc.vector.tensor_add(
                    V[:, tb, ch * 6 : (ch + 1) * 6, 0:D],
                    ps[:, 0:384].rearrange("p (h d) -> p h d", h=6),
                    bv_bc[:, ch * 384 : (ch + 1) * 384].rearrange("p (h d) -> p h d", h=6),
                )

        # ---------- head-pair loop ----------
        # Q^T/K^T projection for pair j+1 is emitted as four psum-group
        # closures interleaved into pair j's attention loop, so the PE queue
        # always has dependency-free matmuls behind each attention sem-wait
        # (hides LDWEIGHTS that otherwise cannot prefetch across a wait).
        def issue_wqk(j):
            wqk = wqk_pool.tile([128, CB, 2, 128], f32r, name="wqk", tag="wqk")
            for cb in range(CB):
                for qk in range(2):
                    nc.sync.dma_start(
                        wqk[:, cb, qk, :],
                        wqkv_d[
                            cb * 128 : (cb + 1) * 128,
                            qk * C + j * 128 : qk * C + (j + 1) * 128,
                        ],
                    )
            return wqk

        def proj_group_emitters(j, wqk, qkt):
            ems = []
            for qk in range(2):
                for tch in range(2):
                    def g(qk=qk, tch=tch):
                        ps = mm_psum.tile([128, 512], f32, name="ps_qk", tag="mm")
                        for cb in range(CB):
                            nc.tensor.matmul(
                                ps[:],
                                wqk[:, cb, qk, :],
                                xT[:, cb, tch * 512 : (tch + 1) * 512],
                                start=(cb == 0),
                                stop=(cb == CB - 1),
                            )
                        m_idx = qk * 6 + j
                        nc.vector.tensor_scalar_add(
                            qkt[:, qk, tch * 512 : (tch + 1) * 512],
                            ps[:],
                            bqk[:, m_idx : m_idx + 1],
                        )
                    ems.append(g)
            return ems

        wqk0 = issue_wqk(0)
        qkt = qkt_pool.tile([128, 2, T], f32r, name="qkt", tag="qkt")
        for g in proj_group_emitters(0, wqk0, qkt):
            g()

        for j in range(6):
            pending = []
            if j < 5:
                wqk_next = issue_wqk(j + 1)
                qkt_next = qkt_pool.tile([128, 2, T], f32r, name="qkt", tag="qkt")
                pending = proj_group_emitters(j + 1, wqk_next, qkt_next)

            for i in range(2):
                h = 2 * j + i
                # O'^T accumulators: one 512-wide group per PSUM bank, as two
                # separate single-bank tiles so each bank's slot frees as soon
                # as its own normalize half has consumed it (the qc=0 half
                # finishes mid-head, unblocking the next head's first PVs).
                ot2 = [
                    o_psum.tile([D + 1, 512], f32, name=f"ot{q}", tag=f"ot{q}")
                    for q in range(2)
                ]
                for kb in range(TB):
                    v0 = kb * 128  # first causally-valid q for this k-block
                    # per-PSUM-bank column spans, start clamped so every
                    # matmul keeps a moving dim >= 256 (fp32r full rate)
                    spans = []
                    for b2 in range(kb // 4, 2):
                        blo = b2 * 512
                        spans.append((min(max(v0, blo), blo + 256), blo + 512))
                    estart = spans[0][0]
                    st = st_psum.tile([128, T], f32, name="st", tag="st")
                    for c0, c1 in spans:
                        nc.tensor.matmul(
                            st[:, c0:c1],
                            qkt[i * 64 : (i + 1) * 64, 1, kb * 128 : (kb + 1) * 128],
                            qkt[i * 64 : (i + 1) * 64, 0, c0:c1],
                            start=True,
                            stop=True,
                        )
                    pt = pt_pool.tile([128, T], f32r, name="pt", tag="pt")
                    nc.scalar.activation(pt[:, estart:T], st[:, estart:T], Exp, scale=0.125)
                    # zero sub-diagonal cols: region [estart, v0+128), valid iff q >= k
                    width = v0 + 128 - estart
                    mask = maskd if width == 128 else maskw
                    nc.vector.tensor_mul(
                        pt[:, estart : estart + width],
                        pt[:, estart : estart + width],
                        mask[:, 0:width],
                    )
                    for qc in range(kb // 4, 2):
                        qlo = qc * 512
                        sq = min(max(v0, qlo), qlo + 256)
                        nc.tensor.matmul(
                            ot2[qc][:, sq - qlo : 512],
                            V[:, kb, h, :],
                            pt[:, sq : qlo + 512],
                            start=(kb == 0),
                            stop=(kb == 4 * qc + 3),
                        )
                    if kb in (2, 5) and pending:
                        pending.pop(0)()
                    if kb == 3 or kb == 7:
                        # the qc2 = kb//4 O' bank just closed (stop at kb =
                        # 4*qc2+3): normalize that half now so only the second
                        # half's chain is exposed at the head boundary.
                        # 1/s = exp(-ln s) on ACT (same pinned table set).
                        qc2 = kb // 4
                        lns = small_pool.tile([1, 512], f32, name="lns", tag="lns")
                        nc.scalar.activation(lns[:], ot2[qc2][D : D + 1, :], Ln)
                        recip = small_pool.tile([1, 512], f32, name="recip", tag="recip")
                        nc.scalar.activation(recip[:], lns[:], Exp, scale=-1.0)
                        rbc = small_pool.tile([64, 512], f32, name="rbc", tag="rbc")
                        nc.gpsimd.partition_broadcast(rbc[:], recip[:])
                        nc.vector.tensor_mul(
                            OT[j][i * 64 : (i + 1) * 64, qc2 * 512 : (qc2 + 1) * 512],
                            ot2[qc2][0:D, :],
                            rbc[:],
                        )

            for g in pending:
                g()
            if j < 5:
                qkt = qkt_next

        # ---------- output projection ----------
        for tb in range(TB):
            yt = y_pool.tile([128, C], f32, name="yt", tag="yt")
            for ch in range(2):
                ps = mm_psum.tile([128, 512], f32, name="ps_y", tag="mm")
                for cb in range(CB):
                    nc.tensor.matmul(
                        ps[:, 0:384],
                        OT[cb][:, tb * 128 : (tb + 1) * 128],
                        wout[:, cb, ch * 384 : (ch + 1) * 384],
                        start=(cb == 0),
                        stop=(cb == CB - 1),
                    )
                nc.vector.tensor_add(
                    yt[:, ch * 384 : (ch + 1) * 384],
                    ps[:, 0:384],
                    bo_bc[:, ch * 384 : (ch + 1) * 384],
                )
            nc.sync.dma_start(y_d[tb * 128 : (tb + 1) * 128, :], yt[:])


def build():
    if "nc" in _CACHE:
        return _CACHE["nc"]
    _ensure_path()
    import concourse.bacc as bacc
    import concourse.mybir as mybir
    import concourse.tile as tile
    from concourse.masks import make_identity

    nc = bacc.Bacc(
        "TRN2",
        target_bir_lowering=False,
        debug=False,
        enable_asserts=False,
        num_devices=NCORES,
    )
    with tile.TileContext(nc) as tc:
        _emit(nc, tc, tile, mybir, make_identity)

    # Both Exp and Ln live in the 'natural_log_exp_and_others' ACT table set,
    # but the table-load pass maps Exp to the first set containing it
    # ('exp_and_others'), so Exp/Ln ping-pong table loads every head
    # (~1.3us each).  Restrict Exp membership to the natural_log set for the
    # duration of compile; dict order (= act_func_set_id) is preserved.
    orig_tables = bacc.get_activation_tables

    def _pinned_tables(arch):
        tables = orig_tables(arch)
        exp_t = mybir.ActivationFunctionType.Exp
        if any(exp_t in fns for name, fns in tables.items() if "natural_log" in name):
            for name, fns in tables.items():
                if "natural_log" not in name:
                    fns.discard(exp_t)
        return tables

    bacc.get_activation_tables = _pinned_tables
    try:
        nc.compile()
    finally:
        bacc.get_activation_tables = orig_tables
    _CACHE["nc"] = nc
    return nc


def _in_maps(x, W_qkv, b_qkv, W_out, b_out):
    x = np.ascontiguousarray(np.asarray(x, dtype=np.float32))
    W_qkv = np.ascontiguousarray(np.asarray(W_qkv, dtype=np.float32))
    b_qkv = np.ascontiguousarray(np.asarray(b_qkv, dtype=np.float32))
    W_out = np.ascontiguousarray(np.asarray(W_out, dtype=np.float32))
    b_out = np.ascontiguousarray(np.asarray(b_out, dtype=np.float32))
    return [
        {
            "x": x[b],
            "W_qkv": W_qkv,
            "b_qkv": b_qkv,
            "W_out": W_out,
            "b_out": b_out,
        }
        for b in range(B)
    ]


def _install_ntff_hook():
    """The image's antenv package lacks axon_hooks; synthesize it so
    run_bass_kernel_spmd(trace=True) can NTFF-profile via libaxon_pjrt.so."""
    import sys
    import types

    if "antenv.axon_hooks" in sys.modules:
        return
    mod = types.ModuleType("antenv.axon_hooks")
    state = {"hook": None}
    mod.set_axon_ntff_profile_hook = lambda h: state.__setitem__("hook", h)
    mod.get_axon_ntff_profile_hook = lambda: state["hook"]
    sys.modules["antenv.axon_hooks"] = mod
    import antenv

    antenv.axon_hooks = mod
    try:
        if "/root/.axon_site" not in sys.path:
            sys.path.append("/root/.axon_site")
        from trn_agent_boot.trn_boot import _ntff_profile_via_ctypes

        mod.set_axon_ntff_profile_hook(
            _ntff_profile_via_ctypes("/opt/axon/libaxon_pjrt.so")
        )
    except Exception as exc:  # degrade to no tracing
        print(f"ntff hook unavailable: {exc}", file=sys.stderr)


def run(x, W_qkv, b_qkv, W_out, b_out, trace=False):
    _ensure_path()
    if trace:
        _install_ntff_hook()
    from concourse.bass_utils import run_bass_kernel_spmd

    nc = build()
    res = run_bass_kernel_spmd(
        nc,
        _in_maps(x, W_qkv, b_qkv, W_out, b_out),
        core_ids=list(range(NCORES)),
        trace=trace,
    )
    y = np.stack([res.results[b]["y_out"] for b in range(B)], axis=0)
    return y.astype(np.float32, copy=False), res


def kernel(x, W_qkv, b_qkv, W_out, b_out):
    y, _ = run(x, W_qkv, b_qkv, W_out, b_out, trace=False)
    return y



# revision 5
# speedup vs baseline: 1.1321x; 1.1321x over previous
"""Causal self-attention (B=8, T=1024, C=768, H=12, Dh=64) on 8 TRN2 NeuronCores.

Sharding: batch data-parallel. Core b computes the full attention block for
batch element b (weights replicated). No collectives.

Per-core dataflow (projections fp32r, attention operands bf16):
  1. x [T,C] -> xT [C,T] via PE transposes (f32r, 1.5 cyc/row), ACT evac.
  2. Q^T,K^T [C,T] = W^T @ xT (f32r mms, evac casts to bf16 qkt);
     V [t, c] = x @ W_v (f32r mms, bias-add evac casts to bf16), stored per
     head with an all-ones bf16 column (V_aug [k, 65]) so the P@V matmul also
     accumulates softmax denominators.
  3. Per head h, software-pipelined over k-blocks: S^T(kb) [k=128, q] emitted
     one block AHEAD of the exp/mask/PV chain so the PE streams back-to-back
     while ACT exps the previous block. P^T = exp(S^T/8) (ACT, bf16 out, exact
     causal spans - bf16 mms run full rate at any N so no >=256 clamping).
     Sub-diagonal zeroing via in-place gpsimd affine_select (no DVE mask mul).
     O'^T [65, q] += V_aug^T @ P^T; row 64 = softmax denominator.
  4. Normalize per 512-wide q-chunk: qc0 on ACT (1/s = exp(-ln s), same pinned
     table), qc1 on DVE (reciprocal_approx_fast) - splits the reciprocal load
     across engines; gpsimd partition_broadcast; DVE multiply -> OT (f32r).
     The last PV + normalize of each head is deferred into the next head's
     pipeline to avoid exposing the exp->PV tail latency on the PE.
  5. y [T,C] = OT-as-lhsT @ W_out + b_out (f32r), DMA to DRAM.
"""

import numpy as np

B, T, C = 8, 1024, 768
H, D = 12, 64
TB = T // 128  # 8 t/k blocks
CB = C // 128  # 6 channel blocks
J = H // 2  # 6 head pairs
NCORES = 8

_CACHE = {}


def _ensure_path():
    import sys

    for p in ("/opt/trn_rl_repo",):
        if p not in sys.path:
            sys.path.insert(0, p)


def _emit(nc, tc, tile, mybir, make_identity):
    f32 = mybir.dt.float32
    f32r = mybir.dt.float32r
    bf16 = mybir.dt.bfloat16
    Exp = mybir.ActivationFunctionType.Exp
    Ln = mybir.ActivationFunctionType.Ln
    isge = mybir.AluOpType.is_ge

    x_d = nc.dram_tensor("x", [T, C], f32, kind="ExternalInput")
    wqkv_d = nc.dram_tensor("W_qkv", [C, 3 * C], f32r, kind="ExternalInput")
    bqkv_d = nc.dram_tensor("b_qkv", [3 * C], f32, kind="ExternalInput")
    wout_d = nc.dram_tensor("W_out", [C, C], f32r, kind="ExternalInput")
    bout_d = nc.dram_tensor("b_out", [C], f32, kind="ExternalInput")
    y_d = nc.dram_tensor("y_out", [T, C], f32, kind="ExternalOutput")

    with (
        tc.tile_pool(name="const", bufs=1) as const_pool,
        tc.tile_pool(name="wres", bufs=1) as wres,
        tc.tile_pool(name="wqkp", bufs=2) as wqk_pool,
        tc.tile_pool(name="xin", bufs=4) as xin_pool,
        tc.tile_pool(name="big", bufs=1) as big,
        tc.tile_pool(name="qktp", bufs=2) as qkt_pool,
        tc.tile_pool(name="ptp", bufs=3) as pt_pool,
        tc.tile_pool(name="yp", bufs=2) as y_pool,
        tc.tile_pool(name="smallp", bufs=2) as small_pool,
        # PSUM: st 2x[128,1024]f32 = 4 banks; ot 3x[*,512]f32 = 3; mm 1 = 1.
        tc.tile_pool(name="stp", bufs=2, space="PSUM") as st_psum,
        tc.tile_pool(name="op", bufs=3, space="PSUM") as o_psum,
        tc.tile_pool(name="mmp", bufs=1, space="PSUM") as mm_psum,
    ):
        ident = const_pool.tile([128, 128], f32, name="ident")
        make_identity(nc, ident[:])

        # W_v first: the V projection is the preamble critical path.
        wv = wres.tile([128, CB, C], f32r, name="wv")
        for cb in range(CB):
            nc.scalar.dma_start(
                wv[:, cb, :], wqkv_d[cb * 128 : (cb + 1) * 128, 2 * C : 3 * C]
            )

        xT = big.tile([128, CB, T], f32r, name="xT")
        V = big.tile([128, TB, H, D + 1], bf16, name="V")
        OT = [big.tile([128, T], f32r, name=f"OT{j}", tag=f"OT{j}") for j in range(J)]

        x_tiles = []
        for tb in range(TB):
            x_in = xin_pool.tile([128, C], f32, name="x_in", tag="x_in", bufs=TB)
            nc.sync.dma_start(x_in[:], x_d[tb * 128 : (tb + 1) * 128, :])
            x_tiles.append(x_in)

        # b_qkv as [128, 18]: column m holds channels m*128..m*128+127
        bqk = const_pool.tile([128, 18], f32, name="bqk")
        nc.scalar.dma_start(bqk[:], bqkv_d[:].rearrange("(m p) -> p m", p=128))

        bv_bc = const_pool.tile([128, C], f32, name="bv_bc")
        nc.scalar.dma_start(bv_bc[0:1, :], bqkv_d[2 * C : 3 * C][None, :])
        nc.gpsimd.partition_broadcast(bv_bc[:], bv_bc[0:1, :])

        bo_bc = const_pool.tile([128, C], f32, name="bo_bc")
        nc.scalar.dma_start(bo_bc[0:1, :], bout_d[:][None, :])
        nc.gpsimd.partition_broadcast(bo_bc[:], bo_bc[0:1, :])

        ones96 = const_pool.tile([128, TB * H], f32, name="ones96")
        nc.gpsimd.memset(ones96[:], 1.0)
        nc.vector.tensor_copy(
            V[:, :, :, D], ones96[:].rearrange("p (t h) -> p t h", t=TB)
        )

        def issue_wqk(j):
            wqk = wqk_pool.tile([128, CB, 2, 128], f32r, name="wqk", tag="wqk")
            for cb in range(CB):
                for qk in range(2):
                    nc.sync.dma_start(
                        wqk[:, cb, qk, :],
                        wqkv_d[
                            cb * 128 : (cb + 1) * 128,
                            qk * C + j * 128 : qk * C + (j + 1) * 128,
                        ],
                    )
            return wqk

        wqk0 = issue_wqk(0)

        wout = wres.tile([128, CB, C], f32r, name="wout")
        for cb in range(CB):
            nc.scalar.dma_start(wout[:, cb, :], wout_d[cb * 128 : (cb + 1) * 128, :])

        def proj_group_emitters(j, wqk, qkt):
            # one group per (qk, t-half): 6-mm f32r chain -> bias-add evac
            # casting to bf16 qkt. tch=0 groups only need xT t<512 (tb 0-3).
            ems = []
            for tch in range(2):
                for qk in range(2):
                    def g(qk=qk, tch=tch):
                        ps = mm_psum.tile([128, 512], f32, name="ps_qk", tag="mm")
                        for cb in range(CB):
                            nc.tensor.matmul(
                                ps[:],
                                wqk[:, cb, qk, :],
                                xT[:, cb, tch * 512 : (tch + 1) * 512],
                                start=(cb == 0),
                                stop=(cb == CB - 1),
                            )
                        m = qk * 6 + j
                        nc.vector.tensor_scalar_add(
                            qkt[:, qk, tch * 512 : (tch + 1) * 512],
                            ps[:],
                            bqk[:, m : m + 1],
                        )
                    ems.append(g)
            return ems

        # ---------- preamble: transpose x, project V, project Q/K pair 0 ----
        qkt = qkt_pool.tile([128, 2, T], bf16, name="qkt", tag="qkt")
        pre_projs = proj_group_emitters(0, wqk0, qkt)

        def v_chain(tb, ch):
            ps = o_psum.tile([128, 512], f32, name="ps_v", tag="ot")
            for cb in range(CB):
                nc.tensor.matmul(
                    ps[:, 0:384],
                    xT[:, cb, tb * 128 : (tb + 1) * 128],
                    wv[:, cb, ch * 384 : (ch + 1) * 384],
                    start=(cb == 0),
                    stop=(cb == CB - 1),
                )
            nc.vector.tensor_add(
                V[:, tb, ch * 6 : (ch + 1) * 6, 0:D],
                ps[:, 0:384].rearrange("p (h d) -> p h d", h=6),
                bv_bc[:, ch * 384 : (ch + 1) * 384].rearrange("p (h d) -> p h d", h=6),
            )

        for tb in range(TB):
            tp = st_psum.tile([128, T], f32, name="tp", tag="st")
            for c in range(CB):
                nc.tensor.transpose(
                    tp[:, c * 128 : (c + 1) * 128],
                    x_tiles[tb][:, c * 128 : (c + 1) * 128],
                    ident[:],
                )
            nc.scalar.copy(
                xT[:, :, tb * 128 : (tb + 1) * 128],
                tp[:, 0:C].rearrange("p (c t) -> p c t", c=CB),
            )
            v_chain(tb, 0)
            v_chain(tb, 1)
            if tb == 3:
                pre_projs[0]()  # (t-half 0, q)
                pre_projs[1]()  # (t-half 0, k)
            if tb == 7:
                pre_projs[2]()
                pre_projs[3]()

        # ---------- attention ----------
        def norm(j, i, ot_qc, qc, variant):
            # OT[j][head-half, qc-chunk] = O'(0:D) * (1 / O'(D)) broadcast
            if variant == "act":
                lns = small_pool.tile([1, 512], f32, name="lns", tag="lns")
                nc.scalar.activation(lns[:], ot_qc[D : D + 1, :], Ln)
                recip = small_pool.tile([1, 512], f32, name="recip", tag="recip")
                nc.scalar.activation(recip[:], lns[:], Exp, scale=-1.0)
            else:
                dn = small_pool.tile([1, 512], f32, name="dn", tag="dn")
                nc.vector.tensor_copy(dn[:], ot_qc[D : D + 1, :])
                recip = small_pool.tile([1, 512], f32, name="recipd", tag="recipd")
                nc.vector.reciprocal_approx_fast(recip[:], dn[:])
            rbc = small_pool.tile([64, 512], f32, name="rbc", tag="rbc")
            nc.gpsimd.partition_broadcast(rbc[:], recip[:])
            nc.vector.tensor_mul(
                OT[j][i * 64 : (i + 1) * 64, qc * 512 : (qc + 1) * 512],
                ot_qc[0:D, :],
                rbc[:],
            )

        deferred = [None]
        for j in range(J):
            if j < J - 1:
                wqk_next = issue_wqk(j + 1)
                qkt_next = qkt_pool.tile([128, 2, T], bf16, name="qkt", tag="qkt")
                pending = proj_group_emitters(j + 1, wqk_next, qkt_next)
            else:
                pending = []

            for i in range(2):
                h = 2 * j + i
                # last pair: no proj filler, keep ACT exp-only (chains on DVE)
                qc0_variant = "dve" if j == J - 1 else "act"
                ot = [
                    o_psum.tile([D + 1, 512], f32, name=f"ot{qc}", tag="ot")
                    for qc in range(2)
                ]
                sts = {}

                def process(pkb, ot=ot, sts=sts, h=h, i=i, j=j, qc0_variant=qc0_variant):
                    # exp -> causal-zero -> PV for k-block pkb (one behind S)
                    pv0 = pkb * 128
                    pt = pt_pool.tile([128, T], bf16, name="pt", tag="pt")
                    nc.scalar.activation(
                        pt[:, pv0:T], sts[pkb][:, pv0:T], Exp, scale=0.125
                    )
                    nc.gpsimd.affine_select(
                        out=pt[:, pv0 : pv0 + 128],
                        in_=pt[:, pv0 : pv0 + 128],
                        compare_op=isge,
                        fill=0.0,
                        base=0,
                        channel_multiplier=-1,
                        pattern=[[1, 128]],
                    )
                    for qc in range(pkb // 4, 2):
                        sq = max(pv0, qc * 512)
                        nc.tensor.matmul(
                            ot[qc][:, sq - qc * 512 : 512],
                            V[:, pkb, h, :],
                            pt[:, sq : (qc + 1) * 512],
                            start=(pkb == 0),
                            stop=(pkb == 3 + 4 * qc),
                        )
                    if pkb == 3:
                        norm(j, i, ot[0], 0, qc0_variant)
                    return pt

                for kb in range(TB):
                    v0 = kb * 128
                    st = st_psum.tile([128, T], f32, name="st", tag="st")
                    kT = qkt[i * 64 : (i + 1) * 64, 1, v0 : v0 + 128]
                    if kb < 4:
                        nc.tensor.matmul(
                            st[:, v0:512], kT,
                            qkt[i * 64 : (i + 1) * 64, 0, v0:512],
                            start=True, stop=True,
                        )
                        nc.tensor.matmul(
                            st[:, 512:T], kT,
                            qkt[i * 64 : (i + 1) * 64, 0, 512:T],
                            start=True, stop=True,
                        )
                    else:
                        nc.tensor.matmul(
                            st[:, v0:T], kT,
                            qkt[i * 64 : (i + 1) * 64, 0, v0:T],
                            start=True, stop=True,
                        )
                    sts[kb] = st
                    if kb == 1 and deferred[0] is not None:
                        deferred[0]()
                        deferred[0] = None
                    if kb >= 1:
                        process(kb - 1)
                    if kb == 2 and pending:
                        pending.pop(0)()
                    if kb == 5 and pending:
                        pending.pop(0)()

                # head tail: exp/mask kb=7 now; its PV + qc1 normalize deferred
                # into the next head's pipeline (hides the exp->PV latency).
                pt7 = pt_pool.tile([128, T], bf16, name="pt", tag="pt")
                nc.scalar.activation(pt7[:, 896:T], sts[7][:, 896:T], Exp, scale=0.125)
                nc.gpsimd.affine_select(
                    out=pt7[:, 896:T],
                    in_=pt7[:, 896:T],
                    compare_op=isge,
                    fill=0.0,
                    base=0,
                    channel_multiplier=-1,
                    pattern=[[1, 128]],
                )

                def make_deferred(j=j, i=i, h=h, ot=ot, pt7=pt7):
                    def d():
                        nc.tensor.matmul(
                            ot[1][:, 384:512],
                            V[:, 7, h, :],
                            pt7[:, 896:T],
                            start=False,
                            stop=True,
                        )
                        norm(j, i, ot[1], 1, "dve")
                    return d

                deferred[0] = make_deferred()

            for g in pending:
                g()
            if j < J - 1:
                qkt = qkt_next

        if deferred[0] is not None:
            deferred[0]()
            deferred[0] = None

        # ---------- output projection ----------
        for tb in range(TB):
            yt = y_pool.tile([128, C], f32, name="yt", tag="yt")
            for ch in range(2):
                ps = o_psum.tile([128, 512], f32, name="ps_y", tag="ot")
                for cb in range(CB):
                    nc.tensor.matmul(
                        ps[:, 0:384],
                        OT[cb][:, tb * 128 : (tb + 1) * 128],
                        wout[:, cb, ch * 384 : (ch + 1) * 384],
                        start=(cb == 0),
                        stop=(cb == CB - 1),
                    )
                nc.vector.tensor_add(
                    yt[:, ch * 384 : (ch + 1) * 384],
                    ps[:, 0:384],
                    bo_bc[:, ch * 384 : (ch + 1) * 384],
                )
            nc.sync.dma_start(y_d[tb * 128 : (tb + 1) * 128, :], yt[:])


def build():
    if "nc" in _CACHE:
        return _CACHE["nc"]
    _ensure_path()
    import concourse.bacc as bacc
    import concourse.mybir as mybir
    import concourse.tile as tile
    from concourse.masks import make_identity

    nc = bacc.Bacc(
        "TRN2",
        target_bir_lowering=False,
        debug=False,
        enable_asserts=False,
        num_devices=NCORES,
    )
    with tile.TileContext(nc) as tc:
        _emit(nc, tc, tile, mybir, make_identity)

    # Both Exp and Ln live in the 'natural_log_exp_and_others' ACT table set,
    # but the table-load pass maps Exp to the first set containing it
    # ('exp_and_others'), so Exp/Ln ping-pong table loads every head
    # (~1.3us each).  Restrict Exp membership to the natural_log set for the
    # duration of compile; dict order (= act_func_set_id) is preserved.
    orig_tables = bacc.get_activation_tables

    def _pinned_tables(arch):
        tables = orig_tables(arch)
        exp_t = mybir.ActivationFunctionType.Exp
        if any(exp_t in fns for name, fns in tables.items() if "natural_log" in name):
            for name, fns in tables.items():
                if "natural_log" not in name:
                    fns.discard(exp_t)
        return tables

    bacc.get_activation_tables = _pinned_tables
    try:
        nc.compile()
    finally:
        bacc.get_activation_tables = orig_tables
    _CACHE["nc"] = nc
    return nc


def _in_maps(x, W_qkv, b_qkv, W_out, b_out):
    x = np.ascontiguousarray(np.asarray(x, dtype=np.float32))
    W_qkv = np.ascontiguousarray(np.asarray(W_qkv, dtype=np.float32))
    b_qkv = np.ascontiguousarray(np.asarray(b_qkv, dtype=np.float32))
    W_out = np.ascontiguousarray(np.asarray(W_out, dtype=np.float32))
    b_out = np.ascontiguousarray(np.asarray(b_out, dtype=np.float32))
    return [
        {
            "x": x[b],
            "W_qkv": W_qkv,
            "b_qkv": b_qkv,
            "W_out": W_out,
            "b_out": b_out,
        }
        for b in range(B)
    ]


def _install_ntff_hook():
    """The image's antenv package lacks axon_hooks; synthesize it so
    run_bass_kernel_spmd(trace=True) can NTFF-profile via libaxon_pjrt.so."""
    import sys
    import types

    if "antenv.axon_hooks" in sys.modules:
        return
    mod = types.ModuleType("antenv.axon_hooks")
    state = {"hook": None}
    mod.set_axon_ntff_profile_hook = lambda h: state.__setitem__("hook", h)
    mod.get_axon_ntff_profile_hook = lambda: state["hook"]
    sys.modules["antenv.axon_hooks"] = mod
    import antenv

    antenv.axon_hooks = mod
    try:
        if "/root/.axon_site" not in sys.path:
            sys.path.append("/root/.axon_site")
        from trn_agent_boot.trn_boot import _ntff_profile_via_ctypes

        mod.set_axon_ntff_profile_hook(
            _ntff_profile_via_ctypes("/opt/axon/libaxon_pjrt.so")
        )
    except Exception as exc:  # degrade to no tracing
        print(f"ntff hook unavailable: {exc}", file=sys.stderr)


def run(x, W_qkv, b_qkv, W_out, b_out, trace=False):
    _ensure_path()
    if trace:
        _install_ntff_hook()
    from concourse.bass_utils import run_bass_kernel_spmd

    nc = build()
    res = run_bass_kernel_spmd(
        nc,
        _in_maps(x, W_qkv, b_qkv, W_out, b_out),
        core_ids=list(range(NCORES)),
        trace=trace,
    )
    y = np.stack([res.results[b]["y_out"] for b in range(B)], axis=0)
    return y.astype(np.float32, copy=False), res


def kernel(x, W_qkv, b_qkv, W_out, b_out):
    y, _ = run(x, W_qkv, b_qkv, W_out, b_out, trace=False)
    return y


# revision 9
# speedup vs baseline: 1.1693x; 1.0329x over previous
"""Causal self-attention (B=8, T=1024, C=768, H=12, Dh=64) on 8 TRN2 NeuronCores.

Sharding: batch data-parallel. Core b computes the full attention block for
batch element b (weights replicated). No collectives.

Per-core dataflow (all matmuls bf16; fp32 weights cast on idle engines so
every stationary operand gets fast bf16 LDWEIGHTS/FWL):
  1. x [T,C] -> bf16 (DVE cast) -> xT [C,T] via PE transposes, ACT evac.
  2. Q^T,K^T [C,T] = W^T @ xT (evac casts to bf16 qkt); V [t, c] = x @ W_v
     (bias-add evac to bf16), stored per head with an all-ones column
     (V_aug [k, 65]) so the P@V matmul also accumulates softmax denominators.
     W_out DMA+cast deferred to mid-attention (preamble is HBM-bound).
  3. Per head h, software-pipelined over k-blocks: S^T(kb) [k=128, q] emitted
     ahead; P^T = exp(S^T/8) (ACT, bf16 out, exact causal spans) one block
     behind; sub-diagonal zeroing via in-place gpsimd affine_select; PV
     (O'^T [65, q] += V_aug^T @ P^T) lags TWO blocks so its mask is always
     ready when the PE reaches it. Row 64 of O' = softmax denominator.
  4. Normalize per 512-wide q-chunk: qc0 on ACT (1/s = exp(-ln s), same pinned
     table), qc1 on DVE (reciprocal_approx_fast); gpsimd partition_broadcast;
     DVE multiply -> OT (bf16). The last two PVs + qc1 normalize of each head
     are deferred into the next head's pipeline.
  5. y [T,C] = OT-as-lhsT @ W_out + b_out, DMA to DRAM.
"""

import numpy as np

B, T, C = 8, 1024, 768
H, D = 12, 64
TB = T // 128  # 8 t/k blocks
CB = C // 128  # 6 channel blocks
J = H // 2  # 6 head pairs
NCORES = 8

_CACHE = {}


def _ensure_path():
    import sys

    for p in ("/opt/trn_rl_repo",):
        if p not in sys.path:
            sys.path.insert(0, p)


def _emit(nc, tc, tile, mybir, make_identity):
    f32 = mybir.dt.float32
    f32r = mybir.dt.float32r
    bf16 = mybir.dt.bfloat16
    Exp = mybir.ActivationFunctionType.Exp
    Ln = mybir.ActivationFunctionType.Ln
    isge = mybir.AluOpType.is_ge

    x_d = nc.dram_tensor("x", [T, C], f32, kind="ExternalInput")
    wqkv_d = nc.dram_tensor("W_qkv", [C, 3 * C], f32r, kind="ExternalInput")
    bqkv_d = nc.dram_tensor("b_qkv", [3 * C], f32, kind="ExternalInput")
    wout_d = nc.dram_tensor("W_out", [C, C], f32r, kind="ExternalInput")
    bout_d = nc.dram_tensor("b_out", [C], f32, kind="ExternalInput")
    y_d = nc.dram_tensor("y_out", [T, C], f32, kind="ExternalOutput")

    with (
        tc.tile_pool(name="const", bufs=1) as const_pool,
        tc.tile_pool(name="wres", bufs=1) as wres,
        tc.tile_pool(name="wqkp", bufs=2) as wqk_pool,
        tc.tile_pool(name="xin", bufs=4) as xin_pool,
        tc.tile_pool(name="big", bufs=1) as big,
        tc.tile_pool(name="qktp", bufs=2) as qkt_pool,
        tc.tile_pool(name="ptp", bufs=3) as pt_pool,
        tc.tile_pool(name="yp", bufs=2) as y_pool,
        tc.tile_pool(name="smallp", bufs=2) as small_pool,
        # PSUM: st 2x[128,1024]f32 = 4 banks; ot 3x[*,512]f32 = 3; mm 1 = 1.
        tc.tile_pool(name="stp", bufs=2, space="PSUM") as st_psum,
        tc.tile_pool(name="op", bufs=3, space="PSUM") as o_psum,
        tc.tile_pool(name="mmp", bufs=1, space="PSUM") as mm_psum,
    ):
        ident = const_pool.tile([128, 128], f32, name="ident")
        make_identity(nc, ident[:])
        ident_bf = const_pool.tile([128, 128], bf16, name="ident_bf")
        nc.vector.tensor_copy(ident_bf[:], ident[:])

        # W_v first: the V projection is the preamble critical path.  Load the
        # ch0 half first so v_chain(tb, 0) can start before the rest lands.
        wv_raw = wres.tile([128, CB, C], f32r, name="wv_raw")
        wv = wres.tile([128, CB, C], bf16, name="wv")
        for half in range(2):
            for cb in range(CB):
                nc.scalar.dma_start(
                    wv_raw[:, cb, half * 384 : (half + 1) * 384],
                    wqkv_d[
                        cb * 128 : (cb + 1) * 128,
                        2 * C + half * 384 : 2 * C + (half + 1) * 384,
                    ],
                )

        xT = big.tile([128, CB, T], bf16, name="xT")
        V = big.tile([128, TB, H, D + 1], bf16, name="V")
        OT = [big.tile([128, T], bf16, name=f"OT{j}", tag=f"OT{j}") for j in range(J)]

        x_tiles = []
        for tb in range(TB):
            x_in = xin_pool.tile([128, C], f32, name="x_in", tag="x_in", bufs=4)
            nc.sync.dma_start(x_in[:], x_d[tb * 128 : (tb + 1) * 128, :])
            x_tiles.append(x_in)

        for half in range(2):
            for cb in range(CB):
                nc.gpsimd.tensor_copy(
                    wv[:, cb, half * 384 : (half + 1) * 384],
                    wv_raw[:, cb, half * 384 : (half + 1) * 384].bitcast(f32),
                )

        # b_qkv as [128, 18]: column m holds channels m*128..m*128+127
        bqk = const_pool.tile([128, 18], f32, name="bqk")
        nc.scalar.dma_start(bqk[:], bqkv_d[:].rearrange("(m p) -> p m", p=128))

        bv_bc = const_pool.tile([128, C], f32, name="bv_bc")
        nc.scalar.dma_start(bv_bc[0:1, :], bqkv_d[2 * C : 3 * C][None, :])
        nc.gpsimd.partition_broadcast(bv_bc[:], bv_bc[0:1, :])

        bo_bc = const_pool.tile([128, C], f32, name="bo_bc")
        nc.scalar.dma_start(bo_bc[0:1, :], bout_d[:][None, :])
        nc.gpsimd.partition_broadcast(bo_bc[:], bo_bc[0:1, :])

        ones96 = const_pool.tile([128, TB * H], f32, name="ones96")
        nc.gpsimd.memset(ones96[:], 1.0)
        nc.vector.tensor_copy(
            V[:, :, :, D], ones96[:].rearrange("p (t h) -> p t h", t=TB)
        )

        def issue_wqk(j):
            wqk_raw = wqk_pool.tile([128, CB, 2, 128], f32r, name="wqk_raw", tag="wqkr")
            for cb in range(CB):
                for qk in range(2):
                    nc.sync.dma_start(
                        wqk_raw[:, cb, qk, :],
                        wqkv_d[
                            cb * 128 : (cb + 1) * 128,
                            qk * C + j * 128 : qk * C + (j + 1) * 128,
                        ],
                    )
            wqk = wqk_pool.tile([128, CB, 2, 128], bf16, name="wqk", tag="wqk")
            for qk in range(2):
                nc.vector.tensor_copy(wqk[:, :, qk, :], wqk_raw[:, :, qk, :].bitcast(f32))
            return wqk

        wqk0 = issue_wqk(0)

        # wout DMA + cast are deferred into the attention phase (the preamble
        # is HBM-bandwidth-bound; wout is not needed until the tail).
        wout_raw = wres.tile([128, CB, C], f32r, name="wout_raw")
        wout = wres.tile([128, CB, C], bf16, name="wout")

        def issue_wout():
            for cb in range(CB):
                nc.scalar.dma_start(
                    wout_raw[:, cb, :], wout_d[cb * 128 : (cb + 1) * 128, :]
                )

        # per-cb cast closures, drip-fed into the gpsimd queue during pairs
        # 3-4 so they don't delay the attention masks (gpsimd is strict FIFO).
        wout_casts = [
            (lambda cb=cb: nc.gpsimd.tensor_copy(wout[:, cb, :], wout_raw[:, cb, :].bitcast(f32)))
            for cb in range(CB)
        ]

        def proj_group_emitters(j, wqk, qkt):
            # one group per (qk, t-half): 6-mm f32r chain -> bias-add evac
            # casting to bf16 qkt. tch=0 groups only need xT t<512 (tb 0-3).
            ems = []
            for tch in range(2):
                for qk in range(2):
                    def g(qk=qk, tch=tch):
                        ps = mm_psum.tile([128, 512], f32, name="ps_qk", tag="mm")
                        for cb in range(CB):
                            nc.tensor.matmul(
                                ps[:],
                                wqk[:, cb, qk, :],
                                xT[:, cb, tch * 512 : (tch + 1) * 512],
                                start=(cb == 0),
                                stop=(cb == CB - 1),
                            )
                        m = qk * 6 + j
                        nc.vector.tensor_scalar_add(
                            qkt[:, qk, tch * 512 : (tch + 1) * 512],
                            ps[:],
                            bqk[:, m : m + 1],
                        )
                    ems.append(g)
            return ems

        # ---------- preamble: transpose x, project V, project Q/K pair 0 ----
        qkt = qkt_pool.tile([128, 2, T], bf16, name="qkt", tag="qkt")
        pre_projs = proj_group_emitters(0, wqk0, qkt)

        def v_chain(tb, ch):
            ps = o_psum.tile([128, 512], f32, name="ps_v", tag="ot")
            for cb in range(CB):
                nc.tensor.matmul(
                    ps[:, 0:384],
                    xT[:, cb, tb * 128 : (tb + 1) * 128],
                    wv[:, cb, ch * 384 : (ch + 1) * 384],
                    start=(cb == 0),
                    stop=(cb == CB - 1),
                )
            nc.vector.tensor_add(
                V[:, tb, ch * 6 : (ch + 1) * 6, 0:D],
                ps[:, 0:384].rearrange("p (h d) -> p h d", h=6),
                bv_bc[:, ch * 384 : (ch + 1) * 384].rearrange("p (h d) -> p h d", h=6),
            )

        for tb in range(TB):
            x_bf = xin_pool.tile([128, C], bf16, name="x_bf", tag="x_bf", bufs=3)
            nc.vector.tensor_copy(x_bf[:], x_tiles[tb][:])
            tp = st_psum.tile([128, T], bf16, name="tp", tag="st")
            for c in range(CB):
                nc.tensor.transpose(
                    tp[:, c * 128 : (c + 1) * 128],
                    x_bf[:, c * 128 : (c + 1) * 128],
                    ident_bf[:],
                )
            nc.scalar.copy(
                xT[:, :, tb * 128 : (tb + 1) * 128],
                tp[:, 0:C].rearrange("p (c t) -> p c t", c=CB),
            )
            v_chain(tb, 0)
            v_chain(tb, 1)
            if tb == 3:
                pre_projs[0]()  # (t-half 0, q)
                pre_projs[1]()  # (t-half 0, k)
            if tb == 7:
                pre_projs[2]()
                pre_projs[3]()

        # ---------- attention ----------
        def norm(j, i, ot_qc, qc, variant):
            # OT[j][head-half, qc-chunk] = O'(0:D) * (1 / O'(D)) broadcast
            if variant == "act":
                lns = small_pool.tile([1, 512], f32, name="lns", tag="lns")
                nc.scalar.activation(lns[:], ot_qc[D : D + 1, :], Ln)
                recip = small_pool.tile([1, 512], f32, name="recip", tag="recip")
                nc.scalar.activation(recip[:], lns[:], Exp, scale=-1.0)
            else:
                dn = small_pool.tile([1, 512], f32, name="dn", tag="dn")
                nc.vector.tensor_copy(dn[:], ot_qc[D : D + 1, :])
                recip = small_pool.tile([1, 512], f32, name="recipd", tag="recipd")
                nc.vector.reciprocal_approx_fast(recip[:], dn[:])
            rbc = small_pool.tile([64, 512], f32, name="rbc", tag="rbc")
            nc.gpsimd.partition_broadcast(rbc[:], recip[:])
            nc.vector.tensor_mul(
                OT[j][i * 64 : (i + 1) * 64, qc * 512 : (qc + 1) * 512],
                ot_qc[0:D, :],
                rbc[:],
            )

        deferred = [None]
        for j in range(J):
            if j < J - 1:
                wqk_next = issue_wqk(j + 1)
                qkt_next = qkt_pool.tile([128, 2, T], bf16, name="qkt", tag="qkt")
                pending = proj_group_emitters(j + 1, wqk_next, qkt_next)
            else:
                pending = []
            if j == 2:
                issue_wout()

            for i in range(2):
                h = 2 * j + i
                # last pair: no proj filler, keep ACT exp-only (chains on DVE)
                qc0_variant = "dve" if j == J - 1 else "act"
                ot = [
                    o_psum.tile([D + 1, 512], f32, name=f"ot{qc}", tag="ot")
                    for qc in range(2)
                ]
                sts = {}
                pts = {}

                def expmask(pkb, sts=sts, pts=pts):
                    # exp -> causal-zero for k-block pkb (one behind S)
                    pv0 = pkb * 128
                    pt = pt_pool.tile([128, T], bf16, name="pt", tag="pt")
                    nc.scalar.activation(
                        pt[:, pv0:T], sts[pkb][:, pv0:T], Exp, scale=0.125
                    )
                    nc.gpsimd.affine_select(
                        out=pt[:, pv0 : pv0 + 128],
                        in_=pt[:, pv0 : pv0 + 128],
                        compare_op=isge,
                        fill=0.0,
                        base=0,
                        channel_multiplier=-1,
                        pattern=[[1, 128]],
                    )
                    pts[pkb] = pt

                def pv(pkb, last, ot=ot, pts=pts, h=h, i=i, j=j, qc0_variant=qc0_variant):
                    # PV for k-block pkb (two behind S: its mask is long done)
                    pv0 = pkb * 128
                    for qc in range(pkb // 4, 2):
                        sq = max(pv0, qc * 512)
                        nc.tensor.matmul(
                            ot[qc][:, sq - qc * 512 : 512],
                            V[:, pkb, h, :],
                            pts[pkb][:, sq : (qc + 1) * 512],
                            start=(pkb == 0),
                            stop=(pkb == 3 + 4 * qc),
                        )
                    if pkb == 3:
                        norm(j, i, ot[0], 0, qc0_variant)
                    if last:
                        norm(j, i, ot[1], 1, "dve")

                for kb in range(TB):
                    v0 = kb * 128
                    st = st_psum.tile([128, T], f32, name="st", tag="st")
                    kT = qkt[i * 64 : (i + 1) * 64, 1, v0 : v0 + 128]
                    if kb < 4:
                        nc.tensor.matmul(
                            st[:, v0:512], kT,
                            qkt[i * 64 : (i + 1) * 64, 0, v0:512],
                            start=True, stop=True,
                        )
                        nc.tensor.matmul(
                            st[:, 512:T], kT,
                            qkt[i * 64 : (i + 1) * 64, 0, 512:T],
                            start=True, stop=True,
                        )
                    else:
                        nc.tensor.matmul(
                            st[:, v0:T], kT,
                            qkt[i * 64 : (i + 1) * 64, 0, v0:T],
                            start=True, stop=True,
                        )
                    sts[kb] = st
                    if kb == 1 and deferred[0] is not None:
                        deferred[0]()
                        deferred[0] = None
                    if kb >= 1:
                        expmask(kb - 1)
                    if kb >= 2:
                        pv(kb - 2, last=False)
                    if kb == 2 and pending:
                        pending.pop(0)()
                    if kb == 5 and pending:
                        pending.pop(0)()
                    if j in (3, 4) and kb in (4, 7) and wout_casts:
                        wout_casts.pop(0)()

                # head tail: exp/mask kb=7 now; PV(6), PV(7) + qc1 normalize
                # deferred into the next head's pipeline (hides exp->PV tail).
                expmask(7)

                def make_deferred(pv=pv):
                    def d():
                        pv(6, last=False)
                        pv(7, last=True)
                    return d

                deferred[0] = make_deferred()

            for g in pending:
                g()
            if j < J - 1:
                qkt = qkt_next

        if deferred[0] is not None:
            deferred[0]()
            deferred[0] = None

        # ---------- output projection ----------
        for tb in range(TB):
            yt = y_pool.tile([128, C], f32, name="yt", tag="yt")
            for ch in range(2):
                ps = o_psum.tile([128, 512], f32, name="ps_y", tag="ot")
                for cb in range(CB):
                    nc.tensor.matmul(
                        ps[:, 0:384],
                        OT[cb][:, tb * 128 : (tb + 1) * 128],
                        wout[:, cb, ch * 384 : (ch + 1) * 384],
                        start=(cb == 0),
                        stop=(cb == CB - 1),
                    )
                nc.vector.tensor_add(
                    yt[:, ch * 384 : (ch + 1) * 384],
                    ps[:, 0:384],
                    bo_bc[:, ch * 384 : (ch + 1) * 384],
                )
            nc.sync.dma_start(y_d[tb * 128 : (tb + 1) * 128, :], yt[:])


def build():
    if "nc" in _CACHE:
        return _CACHE["nc"]
    _ensure_path()
    import concourse.bacc as bacc
    import concourse.mybir as mybir
    import concourse.tile as tile
    from concourse.masks import make_identity

    nc = bacc.Bacc(
        "TRN2",
        target_bir_lowering=False,
        debug=False,
        enable_asserts=False,
        num_devices=NCORES,
    )
    with tile.TileContext(nc) as tc:
        _emit(nc, tc, tile, mybir, make_identity)

    # Both Exp and Ln live in the 'natural_log_exp_and_others' ACT table set,
    # but the table-load pass maps Exp to the first set containing it
    # ('exp_and_others'), so Exp/Ln ping-pong table loads every head
    # (~1.3us each).  Restrict Exp membership to the natural_log set for the
    # duration of compile; dict order (= act_func_set_id) is preserved.
    orig_tables = bacc.get_activation_tables

    def _pinned_tables(arch):
        tables = orig_tables(arch)
        exp_t = mybir.ActivationFunctionType.Exp
        if any(exp_t in fns for name, fns in tables.items() if "natural_log" in name):
            for name, fns in tables.items():
                if "natural_log" not in name:
                    fns.discard(exp_t)
        return tables

    bacc.get_activation_tables = _pinned_tables
    try:
        nc.compile()
    finally:
        bacc.get_activation_tables = orig_tables
    _CACHE["nc"] = nc
    return nc


def _in_maps(x, W_qkv, b_qkv, W_out, b_out):
    x = np.ascontiguousarray(np.asarray(x, dtype=np.float32))
    W_qkv = np.ascontiguousarray(np.asarray(W_qkv, dtype=np.float32))
    b_qkv = np.ascontiguousarray(np.asarray(b_qkv, dtype=np.float32))
    W_out = np.ascontiguousarray(np.asarray(W_out, dtype=np.float32))
    b_out = np.ascontiguousarray(np.asarray(b_out, dtype=np.float32))
    return [
        {
            "x": x[b],
            "W_qkv": W_qkv,
            "b_qkv": b_qkv,
            "W_out": W_out,
            "b_out": b_out,
        }
        for b in range(B)
    ]


def _install_ntff_hook():
    """The image's antenv package lacks axon_hooks; synthesize it so
    run_bass_kernel_spmd(trace=True) can NTFF-profile via libaxon_pjrt.so."""
    import sys
    import types

    if "antenv.axon_hooks" in sys.modules:
        return
    mod = types.ModuleType("antenv.axon_hooks")
    state = {"hook": None}
    mod.set_axon_ntff_profile_hook = lambda h: state.__setitem__("hook", h)
    mod.get_axon_ntff_profile_hook = lambda: state["hook"]
    sys.modules["antenv.axon_hooks"] = mod
    import antenv

    antenv.axon_hooks = mod
    try:
        if "/root/.axon_site" not in sys.path:
            sys.path.append("/root/.axon_site")
        from trn_agent_boot.trn_boot import _ntff_profile_via_ctypes

        mod.set_axon_ntff_profile_hook(
            _ntff_profile_via_ctypes("/opt/axon/libaxon_pjrt.so")
        )
    except Exception as exc:  # degrade to no tracing
        print(f"ntff hook unavailable: {exc}", file=sys.stderr)


def run(x, W_qkv, b_qkv, W_out, b_out, trace=False):
    _ensure_path()
    if trace:
        _install_ntff_hook()
    from concourse.bass_utils import run_bass_kernel_spmd

    nc = build()
    res = run_bass_kernel_spmd(
        nc,
        _in_maps(x, W_qkv, b_qkv, W_out, b_out),
        core_ids=list(range(NCORES)),
        trace=trace,
    )
    y = np.stack([res.results[b]["y_out"] for b in range(B)], axis=0)
    return y.astype(np.float32, copy=False), res


def kernel(x, W_qkv, b_qkv, W_out, b_out):
    y, _ = run(x, W_qkv, b_qkv, W_out, b_out, trace=False)
    return y


# revision 10
# speedup vs baseline: 1.2609x; 1.0784x over previous
"""Causal self-attention (B=8, T=1024, C=768, H=12, Dh=64) on 8 TRN2 NeuronCores.

Sharding: batch data-parallel. Core b computes the full attention block for
batch element b (weights replicated). No collectives.

Per-core dataflow (all matmuls bf16; fp32 weights cast on idle engines so
every stationary operand gets fast bf16 LDWEIGHTS/FWL):
  1. x [T,C] -> bf16 (DVE cast) -> xT [C,T] via PE transposes, ACT evac.
  2. Q^T,K^T [C,T] = W^T @ xT (evac casts to bf16 qkt); V [t, c] = x @ W_v
     (bias-add evac to bf16), stored per head with an all-ones column
     (V_aug [k, 65]) so the P@V matmul also accumulates softmax denominators.
     W_out DMA+cast deferred to mid-attention (preamble is HBM-bound).
  3. Per head h, software-pipelined over k-blocks: S^T(kb) [k=128, q] emitted
     ahead; P^T = exp(S^T/8) (ACT, bf16 out, exact causal spans) one block
     behind; sub-diagonal zeroing via in-place gpsimd affine_select; PV
     (O'^T [65, q] += V_aug^T @ P^T) lags TWO blocks so its mask is always
     ready when the PE reaches it. Row 64 of O' = softmax denominator.
  4. Normalize per 512-wide q-chunk: qc0 on ACT (1/s = exp(-ln s), same pinned
     table), qc1 on DVE (reciprocal_approx_fast); gpsimd partition_broadcast;
     DVE multiply -> OT (bf16). The last two PVs + qc1 normalize of each head
     are deferred into the next head's pipeline.
  5. y [T,C] = OT-as-lhsT @ W_out + b_out, DMA to DRAM.
"""

import numpy as np

B, T, C = 8, 1024, 768
H, D = 12, 64
TB = T // 128  # 8 t/k blocks
CB = C // 128  # 6 channel blocks
J = H // 2  # 6 head pairs
NCORES = 8

_CACHE = {}


def _ensure_path():
    import sys

    for p in ("/opt/trn_rl_repo",):
        if p not in sys.path:
            sys.path.insert(0, p)


def _emit(nc, tc, tile, mybir, make_identity):
    f32 = mybir.dt.float32
    f32r = mybir.dt.float32r
    bf16 = mybir.dt.bfloat16
    Exp = mybir.ActivationFunctionType.Exp
    Ln = mybir.ActivationFunctionType.Ln
    isge = mybir.AluOpType.is_ge

    x_d = nc.dram_tensor("x", [T, C], f32, kind="ExternalInput")
    wqkv_d = nc.dram_tensor("W_qkv", [C, 3 * C], f32r, kind="ExternalInput")
    bqkv_d = nc.dram_tensor("b_qkv", [3 * C], f32, kind="ExternalInput")
    wout_d = nc.dram_tensor("W_out", [C, C], f32r, kind="ExternalInput")
    bout_d = nc.dram_tensor("b_out", [C], f32, kind="ExternalInput")
    y_d = nc.dram_tensor("y_out", [T, C], f32, kind="ExternalOutput")

    with (
        tc.tile_pool(name="const", bufs=1) as const_pool,
        tc.tile_pool(name="wres", bufs=1) as wres,
        tc.tile_pool(name="wqkp", bufs=2) as wqk_pool,
        tc.tile_pool(name="xin", bufs=4) as xin_pool,
        tc.tile_pool(name="big", bufs=1) as big,
        tc.tile_pool(name="qktp", bufs=2) as qkt_pool,
        tc.tile_pool(name="ptp", bufs=3) as pt_pool,
        tc.tile_pool(name="yp", bufs=2) as y_pool,
        tc.tile_pool(name="smallp", bufs=2) as small_pool,
        # PSUM: st 2x[128,1024]f32 = 4 banks; ot 3x[*,512]f32 = 3; mm 1 = 1.
        tc.tile_pool(name="stp", bufs=2, space="PSUM") as st_psum,
        tc.tile_pool(name="op", bufs=3, space="PSUM") as o_psum,
        tc.tile_pool(name="mmp", bufs=1, space="PSUM") as mm_psum,
    ):
        ident = const_pool.tile([128, 128], f32, name="ident")
        make_identity(nc, ident[:])
        ident_bf = const_pool.tile([128, 128], bf16, name="ident_bf")
        nc.vector.tensor_copy(ident_bf[:], ident[:])

        # Weights are loaded via gpsimd SWDGE casting DMAs (f32 in HBM ->
        # bf16 in SBUF): no staging tiles, no engine-time cast ops.  W_v ch0
        # half first - the V projection is the preamble critical path.
        wv = wres.tile([128, CB, C], bf16, name="wv")

        def wv_dma(half):
            nc.gpsimd.dma_start(
                wv[:, :, half * 384 : (half + 1) * 384],
                wqkv_d[:, 2 * C + half * 384 : 2 * C + (half + 1) * 384].rearrange(
                    "(cb p) f -> p cb f", p=128
                ),
            )

        wv_dma(0)

        xT = big.tile([128, CB, T], bf16, name="xT")
        V = big.tile([128, TB, H, D + 1], bf16, name="V")
        OT = [big.tile([128, T], bf16, name=f"OT{j}", tag=f"OT{j}") for j in range(J)]

        x_tiles = []
        for tb in range(TB):
            x_bf = xin_pool.tile([128, C], bf16, name="x_bf", tag="x_bf", bufs=TB)
            nc.gpsimd.dma_start(x_bf[:], x_d[tb * 128 : (tb + 1) * 128, :])
            x_tiles.append(x_bf)
            if tb == 3:
                wv_dma(1)

        # b_qkv as [128, 18]: column m holds channels m*128..m*128+127
        bqk = const_pool.tile([128, 18], f32, name="bqk")
        nc.scalar.dma_start(bqk[:], bqkv_d[:].rearrange("(m p) -> p m", p=128))

        bv_bc = const_pool.tile([128, C], f32, name="bv_bc")
        nc.scalar.dma_start(bv_bc[0:1, :], bqkv_d[2 * C : 3 * C][None, :])
        nc.gpsimd.partition_broadcast(bv_bc[:], bv_bc[0:1, :])

        bo_bc = const_pool.tile([128, C], f32, name="bo_bc")
        nc.scalar.dma_start(bo_bc[0:1, :], bout_d[:][None, :])
        nc.gpsimd.partition_broadcast(bo_bc[:], bo_bc[0:1, :])

        ones96 = const_pool.tile([128, TB * H], f32, name="ones96")
        nc.gpsimd.memset(ones96[:], 1.0)
        nc.vector.tensor_copy(
            V[:, :, :, D], ones96[:].rearrange("p (t h) -> p t h", t=TB)
        )

        def issue_wqk(j):
            wqk = wqk_pool.tile([128, CB, 2, 128], bf16, name="wqk", tag="wqk")
            for qk in range(2):
                nc.gpsimd.dma_start(
                    wqk[:, :, qk, :],
                    wqkv_d[:, qk * C + j * 128 : qk * C + (j + 1) * 128].rearrange(
                        "(cb p) f -> p cb f", p=128
                    ),
                )
            return wqk

        wqk0 = issue_wqk(0)

        # wout DMA is deferred into the attention phase (the preamble is
        # HBM-bandwidth-bound; wout is not needed until the tail).
        wout = wres.tile([128, CB, C], bf16, name="wout")

        def issue_wout():
            nc.gpsimd.dma_start(
                wout[:, :, :], wout_d[:, :].rearrange("(cb p) f -> p cb f", p=128)
            )

        def proj_group_emitters(j, wqk, qkt):
            # one group per (qk, t-half): 6-mm f32r chain -> bias-add evac
            # casting to bf16 qkt. tch=0 groups only need xT t<512 (tb 0-3).
            ems = []
            for tch in range(2):
                for qk in range(2):
                    def g(qk=qk, tch=tch):
                        ps = mm_psum.tile([128, 512], f32, name="ps_qk", tag="mm")
                        for cb in range(CB):
                            nc.tensor.matmul(
                                ps[:],
                                wqk[:, cb, qk, :],
                                xT[:, cb, tch * 512 : (tch + 1) * 512],
                                start=(cb == 0),
                                stop=(cb == CB - 1),
                            )
                        m = qk * 6 + j
                        nc.vector.tensor_scalar_add(
                            qkt[:, qk, tch * 512 : (tch + 1) * 512],
                            ps[:],
                            bqk[:, m : m + 1],
                        )
                    ems.append(g)
            return ems

        # ---------- preamble: transpose x, project V, project Q/K pair 0 ----
        qkt = qkt_pool.tile([128, 2, T], bf16, name="qkt", tag="qkt")
        pre_projs = proj_group_emitters(0, wqk0, qkt)

        def v_chain(tb, ch):
            ps = o_psum.tile([128, 512], f32, name="ps_v", tag="ot")
            for cb in range(CB):
                nc.tensor.matmul(
                    ps[:, 0:384],
                    xT[:, cb, tb * 128 : (tb + 1) * 128],
                    wv[:, cb, ch * 384 : (ch + 1) * 384],
                    start=(cb == 0),
                    stop=(cb == CB - 1),
                )
            nc.vector.tensor_add(
                V[:, tb, ch * 6 : (ch + 1) * 6, 0:D],
                ps[:, 0:384].rearrange("p (h d) -> p h d", h=6),
                bv_bc[:, ch * 384 : (ch + 1) * 384].rearrange("p (h d) -> p h d", h=6),
            )

        for tb in range(TB):
            tp = st_psum.tile([128, T], bf16, name="tp", tag="st")
            for c in range(CB):
                nc.tensor.transpose(
                    tp[:, c * 128 : (c + 1) * 128],
                    x_tiles[tb][:, c * 128 : (c + 1) * 128],
                    ident_bf[:],
                )
            nc.scalar.copy(
                xT[:, :, tb * 128 : (tb + 1) * 128],
                tp[:, 0:C].rearrange("p (c t) -> p c t", c=CB),
            )
            v_chain(tb, 0)
            v_chain(tb, 1)
            if tb == 3:
                pre_projs[0]()  # (t-half 0, q)
                pre_projs[1]()  # (t-half 0, k)
            if tb == 7:
                pre_projs[2]()
                pre_projs[3]()

        # ---------- attention ----------
        def norm(j, i, ot_qc, qc, variant):
            # OT[j][head-half, qc-chunk] = O'(0:D) * (1 / O'(D)) broadcast
            if variant == "act":
                lns = small_pool.tile([1, 512], f32, name="lns", tag="lns")
                nc.scalar.activation(lns[:], ot_qc[D : D + 1, :], Ln)
                recip = small_pool.tile([1, 512], f32, name="recip", tag="recip")
                nc.scalar.activation(recip[:], lns[:], Exp, scale=-1.0)
            else:
                dn = small_pool.tile([1, 512], f32, name="dn", tag="dn")
                nc.vector.tensor_copy(dn[:], ot_qc[D : D + 1, :])
                recip = small_pool.tile([1, 512], f32, name="recipd", tag="recipd")
                nc.vector.reciprocal_approx_fast(recip[:], dn[:])
            rbc = small_pool.tile([64, 512], f32, name="rbc", tag="rbc")
            nc.gpsimd.partition_broadcast(rbc[:], recip[:])
            nc.vector.tensor_mul(
                OT[j][i * 64 : (i + 1) * 64, qc * 512 : (qc + 1) * 512],
                ot_qc[0:D, :],
                rbc[:],
            )

        deferred = [None]
        for j in range(J):
            if j < J - 1:
                wqk_next = issue_wqk(j + 1)
                qkt_next = qkt_pool.tile([128, 2, T], bf16, name="qkt", tag="qkt")
                pending = proj_group_emitters(j + 1, wqk_next, qkt_next)
            else:
                pending = []
            if j == 2:
                issue_wout()

            for i in range(2):
                h = 2 * j + i
                # last pair: no proj filler, keep ACT exp-only (chains on DVE)
                qc0_variant = "dve" if j == J - 1 else "act"
                ot = [
                    o_psum.tile([D + 1, 512], f32, name=f"ot{qc}", tag="ot")
                    for qc in range(2)
                ]
                sts = {}
                pts = {}

                def expmask(pkb, sts=sts, pts=pts):
                    # exp -> causal-zero for k-block pkb (one behind S)
                    pv0 = pkb * 128
                    pt = pt_pool.tile([128, T], bf16, name="pt", tag="pt")
                    nc.scalar.activation(
                        pt[:, pv0:T], sts[pkb][:, pv0:T], Exp, scale=0.125
                    )
                    nc.gpsimd.affine_select(
                        out=pt[:, pv0 : pv0 + 128],
                        in_=pt[:, pv0 : pv0 + 128],
                        compare_op=isge,
                        fill=0.0,
                        base=0,
                        channel_multiplier=-1,
                        pattern=[[1, 128]],
                    )
                    pts[pkb] = pt

                def pv(pkb, last, ot=ot, pts=pts, h=h, i=i, j=j, qc0_variant=qc0_variant):
                    # PV for k-block pkb (two behind S: its mask is long done)
                    pv0 = pkb * 128
                    for qc in range(pkb // 4, 2):
                        sq = max(pv0, qc * 512)
                        nc.tensor.matmul(
                            ot[qc][:, sq - qc * 512 : 512],
                            V[:, pkb, h, :],
                            pts[pkb][:, sq : (qc + 1) * 512],
                            start=(pkb == 0),
                            stop=(pkb == 3 + 4 * qc),
                        )
                    if pkb == 3:
                        norm(j, i, ot[0], 0, qc0_variant)
                    if last:
                        norm(j, i, ot[1], 1, "dve")

                for kb in range(TB):
                    v0 = kb * 128
                    st = st_psum.tile([128, T], f32, name="st", tag="st")
                    kT = qkt[i * 64 : (i + 1) * 64, 1, v0 : v0 + 128]
                    if kb < 4:
                        nc.tensor.matmul(
                            st[:, v0:512], kT,
                            qkt[i * 64 : (i + 1) * 64, 0, v0:512],
                            start=True, stop=True,
                        )
                        nc.tensor.matmul(
                            st[:, 512:T], kT,
                            qkt[i * 64 : (i + 1) * 64, 0, 512:T],
                            start=True, stop=True,
                        )
                    else:
                        nc.tensor.matmul(
                            st[:, v0:T], kT,
                            qkt[i * 64 : (i + 1) * 64, 0, v0:T],
                            start=True, stop=True,
                        )
                    sts[kb] = st
                    if kb == 1 and deferred[0] is not None:
                        deferred[0]()
                        deferred[0] = None
                    if kb >= 1:
                        expmask(kb - 1)
                    if kb >= 2:
                        pv(kb - 2, last=False)
                    if kb == 2 and pending:
                        pending.pop(0)()
                    if kb == 5 and pending:
                        pending.pop(0)()

                # head tail: exp/mask kb=7 now; PV(6), PV(7) + qc1 normalize
                # deferred into the next head's pipeline (hides exp->PV tail).
                expmask(7)

                def make_deferred(pv=pv):
                    def d():
                        pv(6, last=False)
                        pv(7, last=True)
                    return d

                deferred[0] = make_deferred()

            for g in pending:
                g()
            if j < J - 1:
                qkt = qkt_next

        if deferred[0] is not None:
            deferred[0]()
            deferred[0] = None

        # ---------- output projection ----------
        for tb in range(TB):
            yt = y_pool.tile([128, C], f32, name="yt", tag="yt")
            for ch in range(2):
                ps = o_psum.tile([128, 512], f32, name="ps_y", tag="ot")
                for cb in range(CB):
                    nc.tensor.matmul(
                        ps[:, 0:384],
                        OT[cb][:, tb * 128 : (tb + 1) * 128],
                        wout[:, cb, ch * 384 : (ch + 1) * 384],
                        start=(cb == 0),
                        stop=(cb == CB - 1),
                    )
                nc.vector.tensor_add(
                    yt[:, ch * 384 : (ch + 1) * 384],
                    ps[:, 0:384],
                    bo_bc[:, ch * 384 : (ch + 1) * 384],
                )
            nc.sync.dma_start(y_d[tb * 128 : (tb + 1) * 128, :], yt[:])


def build():
    if "nc" in _CACHE:
        return _CACHE["nc"]
    _ensure_path()
    import concourse.bacc as bacc
    import concourse.mybir as mybir
    import concourse.tile as tile
    from concourse.masks import make_identity

    nc = bacc.Bacc(
        "TRN2",
        target_bir_lowering=False,
        debug=False,
        enable_asserts=False,
        num_devices=NCORES,
    )
    with tile.TileContext(nc) as tc:
        _emit(nc, tc, tile, mybir, make_identity)

    # Both Exp and Ln live in the 'natural_log_exp_and_others' ACT table set,
    # but the table-load pass maps Exp to the first set containing it
    # ('exp_and_others'), so Exp/Ln ping-pong table loads every head
    # (~1.3us each).  Restrict Exp membership to the natural_log set for the
    # duration of compile; dict order (= act_func_set_id) is preserved.
    orig_tables = bacc.get_activation_tables

    def _pinned_tables(arch):
        tables = orig_tables(arch)
        exp_t = mybir.ActivationFunctionType.Exp
        if any(exp_t in fns for name, fns in tables.items() if "natural_log" in name):
            for name, fns in tables.items():
                if "natural_log" not in name:
                    fns.discard(exp_t)
        return tables

    bacc.get_activation_tables = _pinned_tables
    try:
        nc.compile()
    finally:
        bacc.get_activation_tables = orig_tables
    _CACHE["nc"] = nc
    return nc


def _in_maps(x, W_qkv, b_qkv, W_out, b_out):
    x = np.ascontiguousarray(np.asarray(x, dtype=np.float32))
    W_qkv = np.ascontiguousarray(np.asarray(W_qkv, dtype=np.float32))
    b_qkv = np.ascontiguousarray(np.asarray(b_qkv, dtype=np.float32))
    W_out = np.ascontiguousarray(np.asarray(W_out, dtype=np.float32))
    b_out = np.ascontiguousarray(np.asarray(b_out, dtype=np.float32))
    return [
        {
            "x": x[b],
            "W_qkv": W_qkv,
            "b_qkv": b_qkv,
            "W_out": W_out,
            "b_out": b_out,
        }
        for b in range(B)
    ]


def _install_ntff_hook():
    """The image's antenv package lacks axon_hooks; synthesize it so
    run_bass_kernel_spmd(trace=True) can NTFF-profile via libaxon_pjrt.so."""
    import sys
    import types

    if "antenv.axon_hooks" in sys.modules:
        return
    mod = types.ModuleType("antenv.axon_hooks")
    state = {"hook": None}
    mod.set_axon_ntff_profile_hook = lambda h: state.__setitem__("hook", h)
    mod.get_axon_ntff_profile_hook = lambda: state["hook"]
    sys.modules["antenv.axon_hooks"] = mod
    import antenv

    antenv.axon_hooks = mod
    try:
        if "/root/.axon_site" not in sys.path:
            sys.path.append("/root/.axon_site")
        from trn_agent_boot.trn_boot import _ntff_profile_via_ctypes

        mod.set_axon_ntff_profile_hook(
            _ntff_profile_via_ctypes("/opt/axon/libaxon_pjrt.so")
        )
    except Exception as exc:  # degrade to no tracing
        print(f"ntff hook unavailable: {exc}", file=sys.stderr)


def run(x, W_qkv, b_qkv, W_out, b_out, trace=False):
    _ensure_path()
    if trace:
        _install_ntff_hook()
    from concourse.bass_utils import run_bass_kernel_spmd

    nc = build()
    res = run_bass_kernel_spmd(
        nc,
        _in_maps(x, W_qkv, b_qkv, W_out, b_out),
        core_ids=list(range(NCORES)),
        trace=trace,
    )
    y = np.stack([res.results[b]["y_out"] for b in range(B)], axis=0)
    return y.astype(np.float32, copy=False), res


def kernel(x, W_qkv, b_qkv, W_out, b_out):
    y, _ = run(x, W_qkv, b_qkv, W_out, b_out, trace=False)
    return y


# revision 11
# speedup vs baseline: 1.2618x; 1.0007x over previous
"""Causal self-attention (B=8, T=1024, C=768, H=12, Dh=64) on 8 TRN2 NeuronCores.

Sharding: batch data-parallel. Core b computes the full attention block for
batch element b (weights replicated). No collectives.

Per-core dataflow (all matmuls bf16; fp32 weights cast on idle engines so
every stationary operand gets fast bf16 LDWEIGHTS/FWL):
  1. x [T,C] -> bf16 (DVE cast) -> xT [C,T] via PE transposes, ACT evac.
  2. Q^T,K^T [C,T] = W^T @ xT (evac casts to bf16 qkt); V [t, c] = x @ W_v
     (bias-add evac to bf16), stored per head with an all-ones column
     (V_aug [k, 65]) so the P@V matmul also accumulates softmax denominators.
     W_out DMA+cast deferred to mid-attention (preamble is HBM-bound).
  3. Per head h, software-pipelined over k-blocks: S^T(kb) [k=128, q] emitted
     ahead; P^T = exp(S^T/8) (ACT, bf16 out, exact causal spans) one block
     behind; sub-diagonal zeroing via in-place gpsimd affine_select; PV
     (O'^T [65, q] += V_aug^T @ P^T) lags TWO blocks so its mask is always
     ready when the PE reaches it. Row 64 of O' = softmax denominator.
  4. Normalize per 512-wide q-chunk: qc0 on ACT (1/s = exp(-ln s), same pinned
     table), qc1 on DVE (reciprocal_approx_fast); gpsimd partition_broadcast;
     DVE multiply -> OT (bf16). The last two PVs + qc1 normalize of each head
     are deferred into the next head's pipeline.
  5. y [T,C] = OT-as-lhsT @ W_out + b_out, DMA to DRAM.
"""

import numpy as np

B, T, C = 8, 1024, 768
H, D = 12, 64
TB = T // 128  # 8 t/k blocks
CB = C // 128  # 6 channel blocks
J = H // 2  # 6 head pairs
NCORES = 8

_CACHE = {}


def _ensure_path():
    import sys

    for p in ("/opt/trn_rl_repo",):
        if p not in sys.path:
            sys.path.insert(0, p)


def _emit(nc, tc, tile, mybir, make_identity):
    f32 = mybir.dt.float32
    f32r = mybir.dt.float32r
    bf16 = mybir.dt.bfloat16
    Exp = mybir.ActivationFunctionType.Exp
    Ln = mybir.ActivationFunctionType.Ln
    isge = mybir.AluOpType.is_ge

    x_d = nc.dram_tensor("x", [T, C], f32, kind="ExternalInput")
    wqkv_d = nc.dram_tensor("W_qkv", [C, 3 * C], f32r, kind="ExternalInput")
    bqkv_d = nc.dram_tensor("b_qkv", [3 * C], f32, kind="ExternalInput")
    wout_d = nc.dram_tensor("W_out", [C, C], f32r, kind="ExternalInput")
    bout_d = nc.dram_tensor("b_out", [C], f32, kind="ExternalInput")
    y_d = nc.dram_tensor("y_out", [T, C], f32, kind="ExternalOutput")

    with (
        tc.tile_pool(name="const", bufs=1) as const_pool,
        tc.tile_pool(name="wres", bufs=1) as wres,
        tc.tile_pool(name="wqkp", bufs=2) as wqk_pool,
        tc.tile_pool(name="xin", bufs=4) as xin_pool,
        tc.tile_pool(name="big", bufs=1) as big,
        tc.tile_pool(name="qktp", bufs=2) as qkt_pool,
        tc.tile_pool(name="ptp", bufs=3) as pt_pool,
        tc.tile_pool(name="yp", bufs=2) as y_pool,
        tc.tile_pool(name="smallp", bufs=2) as small_pool,
        # PSUM: st 2x[128,1024]f32 = 4 banks; ot 3x[*,512]f32 = 3; mm 1 = 1.
        tc.tile_pool(name="stp", bufs=2, space="PSUM") as st_psum,
        tc.tile_pool(name="op", bufs=3, space="PSUM") as o_psum,
        tc.tile_pool(name="mmp", bufs=1, space="PSUM") as mm_psum,
    ):
        ident = const_pool.tile([128, 128], f32, name="ident")
        make_identity(nc, ident[:])
        ident_bf = const_pool.tile([128, 128], bf16, name="ident_bf")
        nc.vector.tensor_copy(ident_bf[:], ident[:])

        # Weights are loaded via gpsimd SWDGE casting DMAs (f32 in HBM ->
        # bf16 in SBUF): no staging tiles, no engine-time cast ops.  The x
        # tiles go first (transposes gate the whole preamble), wv ch0 next.
        wv = wres.tile([128, CB, C], bf16, name="wv")

        def wv_dma(half):
            nc.gpsimd.dma_start(
                wv[:, :, half * 384 : (half + 1) * 384],
                wqkv_d[:, 2 * C + half * 384 : 2 * C + (half + 1) * 384].rearrange(
                    "(cb p) f -> p cb f", p=128
                ),
            )

        xT = big.tile([128, CB, T], bf16, name="xT")
        V = big.tile([128, TB, H, D + 1], bf16, name="V")
        OT = [big.tile([128, T], bf16, name=f"OT{j}", tag=f"OT{j}") for j in range(J)]

        # bias DMAs early (HWDGE, cheap); their gpsimd broadcasts come later.
        bqk = const_pool.tile([128, 18], f32, name="bqk")
        nc.scalar.dma_start(bqk[:], bqkv_d[:].rearrange("(m p) -> p m", p=128))
        bv_bc = const_pool.tile([128, C], f32, name="bv_bc")
        nc.scalar.dma_start(bv_bc[0:1, :], bqkv_d[2 * C : 3 * C][None, :])
        bo_bc = const_pool.tile([128, C], f32, name="bo_bc")
        nc.scalar.dma_start(bo_bc[0:1, :], bout_d[:][None, :])

        x_tiles = []
        for tb in range(TB):
            x_bf = xin_pool.tile([128, C], bf16, name="x_bf", tag="x_bf", bufs=TB)
            nc.gpsimd.dma_start(x_bf[:], x_d[tb * 128 : (tb + 1) * 128, :])
            x_tiles.append(x_bf)
            if tb == 1:
                wv_dma(0)
            if tb == 3:
                nc.gpsimd.partition_broadcast(bv_bc[:], bv_bc[0:1, :])
            if tb == 5:
                wv_dma(1)

        def issue_wqk(j):
            wqk = wqk_pool.tile([128, CB, 2, 128], bf16, name="wqk", tag="wqk")
            for qk in range(2):
                nc.gpsimd.dma_start(
                    wqk[:, :, qk, :],
                    wqkv_d[:, qk * C + j * 128 : qk * C + (j + 1) * 128].rearrange(
                        "(cb p) f -> p cb f", p=128
                    ),
                )
            return wqk

        wqk0 = issue_wqk(0)

        ones96 = const_pool.tile([128, TB * H], f32, name="ones96")
        nc.gpsimd.memset(ones96[:], 1.0)
        nc.vector.tensor_copy(
            V[:, :, :, D], ones96[:].rearrange("p (t h) -> p t h", t=TB)
        )
        nc.gpsimd.partition_broadcast(bo_bc[:], bo_bc[0:1, :])

        # wout DMA is deferred into the attention phase (the preamble is
        # HBM-bandwidth-bound; wout is not needed until the tail).
        wout = wres.tile([128, CB, C], bf16, name="wout")

        def issue_wout():
            nc.gpsimd.dma_start(
                wout[:, :, :], wout_d[:, :].rearrange("(cb p) f -> p cb f", p=128)
            )

        def proj_group_emitters(j, wqk, qkt):
            # one group per (qk, t-half): 6-mm f32r chain -> bias-add evac
            # casting to bf16 qkt. tch=0 groups only need xT t<512 (tb 0-3).
            ems = []
            for tch in range(2):
                for qk in range(2):
                    def g(qk=qk, tch=tch):
                        ps = mm_psum.tile([128, 512], f32, name="ps_qk", tag="mm")
                        for cb in range(CB):
                            nc.tensor.matmul(
                                ps[:],
                                wqk[:, cb, qk, :],
                                xT[:, cb, tch * 512 : (tch + 1) * 512],
                                start=(cb == 0),
                                stop=(cb == CB - 1),
                            )
                        m = qk * 6 + j
                        nc.vector.tensor_scalar_add(
                            qkt[:, qk, tch * 512 : (tch + 1) * 512],
                            ps[:],
                            bqk[:, m : m + 1],
                        )
                    ems.append(g)
            return ems

        # ---------- preamble: transpose x, project V, project Q/K pair 0 ----
        qkt = qkt_pool.tile([128, 2, T], bf16, name="qkt", tag="qkt")
        pre_projs = proj_group_emitters(0, wqk0, qkt)

        def v_chain(tb, ch):
            ps = o_psum.tile([128, 512], f32, name="ps_v", tag="ot")
            for cb in range(CB):
                nc.tensor.matmul(
                    ps[:, 0:384],
                    xT[:, cb, tb * 128 : (tb + 1) * 128],
                    wv[:, cb, ch * 384 : (ch + 1) * 384],
                    start=(cb == 0),
                    stop=(cb == CB - 1),
                )
            nc.vector.tensor_add(
                V[:, tb, ch * 6 : (ch + 1) * 6, 0:D],
                ps[:, 0:384].rearrange("p (h d) -> p h d", h=6),
                bv_bc[:, ch * 384 : (ch + 1) * 384].rearrange("p (h d) -> p h d", h=6),
            )

        for tb in range(TB):
            tp = st_psum.tile([128, T], bf16, name="tp", tag="st")
            for c in range(CB):
                nc.tensor.transpose(
                    tp[:, c * 128 : (c + 1) * 128],
                    x_tiles[tb][:, c * 128 : (c + 1) * 128],
                    ident_bf[:],
                )
            eng = nc.scalar if tb % 2 == 0 else nc.vector
            if tb % 2 == 0:
                nc.scalar.copy(
                    xT[:, :, tb * 128 : (tb + 1) * 128],
                    tp[:, 0:C].rearrange("p (c t) -> p c t", c=CB),
                )
            else:
                nc.vector.tensor_copy(
                    xT[:, :, tb * 128 : (tb + 1) * 128],
                    tp[:, 0:C].rearrange("p (c t) -> p c t", c=CB),
                )
            v_chain(tb, 0)
            v_chain(tb, 1)
            if tb == 3:
                pre_projs[0]()  # (t-half 0, q)
                pre_projs[1]()  # (t-half 0, k)
            if tb == 7:
                pre_projs[2]()
                pre_projs[3]()

        # ---------- attention ----------
        def norm(j, i, ot_qc, qc, variant):
            # OT[j][head-half, qc-chunk] = O'(0:D) * (1 / O'(D)) broadcast
            if variant == "act":
                lns = small_pool.tile([1, 512], f32, name="lns", tag="lns")
                nc.scalar.activation(lns[:], ot_qc[D : D + 1, :], Ln)
                recip = small_pool.tile([1, 512], f32, name="recip", tag="recip")
                nc.scalar.activation(recip[:], lns[:], Exp, scale=-1.0)
            else:
                dn = small_pool.tile([1, 512], f32, name="dn", tag="dn")
                nc.vector.tensor_copy(dn[:], ot_qc[D : D + 1, :])
                recip = small_pool.tile([1, 512], f32, name="recipd", tag="recipd")
                nc.vector.reciprocal_approx_fast(recip[:], dn[:])
            rbc = small_pool.tile([64, 512], f32, name="rbc", tag="rbc")
            nc.gpsimd.partition_broadcast(rbc[:], recip[:])
            nc.vector.tensor_mul(
                OT[j][i * 64 : (i + 1) * 64, qc * 512 : (qc + 1) * 512],
                ot_qc[0:D, :],
                rbc[:],
            )

        deferred = [None]
        for j in range(J):
            if j < J - 1:
                wqk_next = issue_wqk(j + 1)
                qkt_next = qkt_pool.tile([128, 2, T], bf16, name="qkt", tag="qkt")
                pending = proj_group_emitters(j + 1, wqk_next, qkt_next)
            else:
                pending = []
            if j == 2:
                issue_wout()

            for i in range(2):
                h = 2 * j + i
                # last pair: no proj filler, keep ACT exp-only (chains on DVE)
                qc0_variant = "dve" if j == J - 1 else "act"
                ot = [
                    o_psum.tile([D + 1, 512], f32, name=f"ot{qc}", tag="ot")
                    for qc in range(2)
                ]
                sts = {}
                pts = {}

                def expmask(pkb, sts=sts, pts=pts):
                    # exp -> causal-zero for k-block pkb (one behind S)
                    pv0 = pkb * 128
                    pt = pt_pool.tile([128, T], bf16, name="pt", tag="pt")
                    nc.scalar.activation(
                        pt[:, pv0:T], sts[pkb][:, pv0:T], Exp, scale=0.125
                    )
                    nc.gpsimd.affine_select(
                        out=pt[:, pv0 : pv0 + 128],
                        in_=pt[:, pv0 : pv0 + 128],
                        compare_op=isge,
                        fill=0.0,
                        base=0,
                        channel_multiplier=-1,
                        pattern=[[1, 128]],
                    )
                    pts[pkb] = pt

                def pv(pkb, last, ot=ot, pts=pts, h=h, i=i, j=j, qc0_variant=qc0_variant):
                    # PV for k-block pkb (two behind S: its mask is long done)
                    pv0 = pkb * 128
                    for qc in range(pkb // 4, 2):
                        sq = max(pv0, qc * 512)
                        nc.tensor.matmul(
                            ot[qc][:, sq - qc * 512 : 512],
                            V[:, pkb, h, :],
                            pts[pkb][:, sq : (qc + 1) * 512],
                            start=(pkb == 0),
                            stop=(pkb == 3 + 4 * qc),
                        )
                    if pkb == 3:
                        norm(j, i, ot[0], 0, qc0_variant)
                    if last:
                        norm(j, i, ot[1], 1, "dve")

                for kb in range(TB):
                    v0 = kb * 128
                    st = st_psum.tile([128, T], f32, name="st", tag="st")
                    kT = qkt[i * 64 : (i + 1) * 64, 1, v0 : v0 + 128]
                    if kb < 4:
                        nc.tensor.matmul(
                            st[:, v0:512], kT,
                            qkt[i * 64 : (i + 1) * 64, 0, v0:512],
                            start=True, stop=True,
                        )
                        nc.tensor.matmul(
                            st[:, 512:T], kT,
                            qkt[i * 64 : (i + 1) * 64, 0, 512:T],
                            start=True, stop=True,
                        )
                    else:
                        nc.tensor.matmul(
                            st[:, v0:T], kT,
                            qkt[i * 64 : (i + 1) * 64, 0, v0:T],
                            start=True, stop=True,
                        )
                    sts[kb] = st
                    if kb == 1 and deferred[0] is not None:
                        deferred[0]()
                        deferred[0] = None
                    if kb >= 1:
                        expmask(kb - 1)
                    if kb >= 2:
                        pv(kb - 2, last=False)
                    if kb == 2 and pending:
                        pending.pop(0)()
                    if kb == 5 and pending:
                        pending.pop(0)()

                # head tail: exp/mask kb=7 now; PV(6), PV(7) + qc1 normalize
                # deferred into the next head's pipeline (hides exp->PV tail).
                expmask(7)

                def make_deferred(pv=pv):
                    def d():
                        pv(6, last=False)
                        pv(7, last=True)
                    return d

                deferred[0] = make_deferred()

            for g in pending:
                g()
            if j < J - 1:
                qkt = qkt_next

        if deferred[0] is not None:
            deferred[0]()
            deferred[0] = None

        # ---------- output projection ----------
        for tb in range(TB):
            yt = y_pool.tile([128, C], f32, name="yt", tag="yt")
            for ch in range(2):
                ps = o_psum.tile([128, 512], f32, name="ps_y", tag="ot")
                for cb in range(CB):
                    nc.tensor.matmul(
                        ps[:, 0:384],
                        OT[cb][:, tb * 128 : (tb + 1) * 128],
                        wout[:, cb, ch * 384 : (ch + 1) * 384],
                        start=(cb == 0),
                        stop=(cb == CB - 1),
                    )
                nc.vector.tensor_add(
                    yt[:, ch * 384 : (ch + 1) * 384],
                    ps[:, 0:384],
                    bo_bc[:, ch * 384 : (ch + 1) * 384],
                )
            nc.sync.dma_start(y_d[tb * 128 : (tb + 1) * 128, :], yt[:])


def build():
    if "nc" in _CACHE:
        return _CACHE["nc"]
    _ensure_path()
    import concourse.bacc as bacc
    import concourse.mybir as mybir
    import concourse.tile as tile
    from concourse.masks import make_identity

    nc = bacc.Bacc(
        "TRN2",
        target_bir_lowering=False,
        debug=False,
        enable_asserts=False,
        num_devices=NCORES,
    )
    with tile.TileContext(nc) as tc:
        _emit(nc, tc, tile, mybir, make_identity)

    # Both Exp and Ln live in the 'natural_log_exp_and_others' ACT table set,
    # but the table-load pass maps Exp to the first set containing it
    # ('exp_and_others'), so Exp/Ln ping-pong table loads every head
    # (~1.3us each).  Restrict Exp membership to the natural_log set for the
    # duration of compile; dict order (= act_func_set_id) is preserved.
    orig_tables = bacc.get_activation_tables

    def _pinned_tables(arch):
        tables = orig_tables(arch)
        exp_t = mybir.ActivationFunctionType.Exp
        if any(exp_t in fns for name, fns in tables.items() if "natural_log" in name):
            for name, fns in tables.items():
                if "natural_log" not in name:
                    fns.discard(exp_t)
        return tables

    bacc.get_activation_tables = _pinned_tables
    try:
        nc.compile()
    finally:
        bacc.get_activation_tables = orig_tables
    _CACHE["nc"] = nc
    return nc


def _in_maps(x, W_qkv, b_qkv, W_out, b_out):
    x = np.ascontiguousarray(np.asarray(x, dtype=np.float32))
    W_qkv = np.ascontiguousarray(np.asarray(W_qkv, dtype=np.float32))
    b_qkv = np.ascontiguousarray(np.asarray(b_qkv, dtype=np.float32))
    W_out = np.ascontiguousarray(np.asarray(W_out, dtype=np.float32))
    b_out = np.ascontiguousarray(np.asarray(b_out, dtype=np.float32))
    return [
        {
            "x": x[b],
            "W_qkv": W_qkv,
            "b_qkv": b_qkv,
            "W_out": W_out,
            "b_out": b_out,
        }
        for b in range(B)
    ]


def _install_ntff_hook():
    """The image's antenv package lacks axon_hooks; synthesize it so
    run_bass_kernel_spmd(trace=True) can NTFF-profile via libaxon_pjrt.so."""
    import sys
    import types

    if "antenv.axon_hooks" in sys.modules:
        return
    mod = types.ModuleType("antenv.axon_hooks")
    state = {"hook": None}
    mod.set_axon_ntff_profile_hook = lambda h: state.__setitem__("hook", h)
    mod.get_axon_ntff_profile_hook = lambda: state["hook"]
    sys.modules["antenv.axon_hooks"] = mod
    import antenv

    antenv.axon_hooks = mod
    try:
        if "/root/.axon_site" not in sys.path:
            sys.path.append("/root/.axon_site")
        from trn_agent_boot.trn_boot import _ntff_profile_via_ctypes

        mod.set_axon_ntff_profile_hook(
            _ntff_profile_via_ctypes("/opt/axon/libaxon_pjrt.so")
        )
    except Exception as exc:  # degrade to no tracing
        print(f"ntff hook unavailable: {exc}", file=sys.stderr)


def run(x, W_qkv, b_qkv, W_out, b_out, trace=False):
    _ensure_path()
    if trace:
        _install_ntff_hook()
    from concourse.bass_utils import run_bass_kernel_spmd

    nc = build()
    res = run_bass_kernel_spmd(
        nc,
        _in_maps(x, W_qkv, b_qkv, W_out, b_out),
        core_ids=list(range(NCORES)),
        trace=trace,
    )
    y = np.stack([res.results[b]["y_out"] for b in range(B)], axis=0)
    return y.astype(np.float32, copy=False), res


def kernel(x, W_qkv, b_qkv, W_out, b_out):
    y, _ = run(x, W_qkv, b_qkv, W_out, b_out, trace=False)
    return y


# revision 14
# speedup vs baseline: 1.3434x; 1.0647x over previous
"""Causal self-attention (B=8, T=1024, C=768, H=12, Dh=64) on 8 TRN2 NeuronCores.

Sharding: batch data-parallel. Core b computes the full attention block for
batch element b (weights replicated). No collectives.

Per-core dataflow (all matmuls bf16; fp32 weights cast on idle engines so
every stationary operand gets fast bf16 LDWEIGHTS/FWL):
  1. x [T,C] -> bf16 (DVE cast) -> xT [C,T] via PE transposes, ACT evac.
  2. Q^T,K^T [C,T] = W^T @ xT (evac casts to bf16 qkt); V [t, c] = x @ W_v
     (bias-add evac to bf16), stored per head with an all-ones column
     (V_aug [k, 65]) so the P@V matmul also accumulates softmax denominators.
     W_out DMA+cast deferred to mid-attention (preamble is HBM-bound).
  3. Per head h, software-pipelined over k-blocks: S^T(kb) [k=128, q] emitted
     ahead; P^T = exp(S^T/8) (ACT, bf16 out, exact causal spans) one block
     behind; sub-diagonal zeroing via in-place gpsimd affine_select; PV
     (O'^T [65, q] += V_aug^T @ P^T) lags TWO blocks so its mask is always
     ready when the PE reaches it. Row 64 of O' = softmax denominator.
  4. Normalize per 512-wide q-chunk: qc0 on ACT (1/s = exp(-ln s), same pinned
     table), qc1 on DVE (reciprocal_approx_fast); gpsimd partition_broadcast;
     DVE multiply -> OT (bf16). The last two PVs + qc1 normalize of each head
     are deferred into the next head's pipeline.
  5. y [T,C] = OT-as-lhsT @ W_out + b_out, DMA to DRAM.
"""

import numpy as np

B, T, C = 8, 1024, 768
H, D = 12, 64
TB = T // 128  # 8 t/k blocks
CB = C // 128  # 6 channel blocks
J = H // 2  # 6 head pairs
NCORES = 8

_CACHE = {}


def _ensure_path():
    import sys

    for p in ("/opt/trn_rl_repo",):
        if p not in sys.path:
            sys.path.insert(0, p)


def _emit(nc, tc, tile, mybir, make_identity):
    f32 = mybir.dt.float32
    f32r = mybir.dt.float32r
    bf16 = mybir.dt.bfloat16
    Exp = mybir.ActivationFunctionType.Exp
    Ln = mybir.ActivationFunctionType.Ln
    isge = mybir.AluOpType.is_ge

    x_d = nc.dram_tensor("x", [T, C], f32, kind="ExternalInput")
    wqkv_d = nc.dram_tensor("W_qkv", [C, 3 * C], f32r, kind="ExternalInput")
    bqkv_d = nc.dram_tensor("b_qkv", [3 * C], f32, kind="ExternalInput")
    wout_d = nc.dram_tensor("W_out", [C, C], f32r, kind="ExternalInput")
    bout_d = nc.dram_tensor("b_out", [C], f32, kind="ExternalInput")
    y_d = nc.dram_tensor("y_out", [T, C], f32, kind="ExternalOutput")

    with (
        tc.tile_pool(name="const", bufs=1) as const_pool,
        tc.tile_pool(name="wres", bufs=1) as wres,
        tc.tile_pool(name="wqkp", bufs=2) as wqk_pool,
        tc.tile_pool(name="xin", bufs=4) as xin_pool,
        tc.tile_pool(name="big", bufs=1) as big,
        tc.tile_pool(name="qktp", bufs=2) as qkt_pool,
        tc.tile_pool(name="ptp", bufs=6) as pt_pool,
        tc.tile_pool(name="yp", bufs=2) as y_pool,
        tc.tile_pool(name="smallp", bufs=2) as small_pool,
        # PSUM: st 4x[128,512]f32 = 4 banks (S chunks + proj chains +
        # transposes share the rotation); ot 4x[*,512]f32 = 4 banks.
        tc.tile_pool(name="stp", bufs=4, space="PSUM") as st_psum,
        tc.tile_pool(name="op", bufs=4, space="PSUM") as o_psum,
    ):
        ident = const_pool.tile([128, 128], f32, name="ident")
        make_identity(nc, ident[:])
        ident_bf = const_pool.tile([128, 128], bf16, name="ident_bf")
        nc.vector.tensor_copy(ident_bf[:], ident[:])

        # Weights are loaded via gpsimd SWDGE casting DMAs (f32 in HBM ->
        # bf16 in SBUF): no staging tiles, no engine-time cast ops.  The x
        # tiles go first (transposes gate the whole preamble), wv ch0 next.
        wv = wres.tile([128, CB, C], bf16, name="wv")

        def wv_dma(half):
            nc.gpsimd.dma_start(
                wv[:, :, half * 384 : (half + 1) * 384],
                wqkv_d[:, 2 * C + half * 384 : 2 * C + (half + 1) * 384].rearrange(
                    "(cb p) f -> p cb f", p=128
                ),
            )

        xT = big.tile([128, CB, T], bf16, name="xT")
        V = big.tile([128, TB, H, D + 1], bf16, name="V")
        OT = [big.tile([128, T], bf16, name=f"OT{j}", tag=f"OT{j}") for j in range(J)]

        # bias DMAs early (HWDGE, cheap); their gpsimd broadcasts come later.
        # b_qkv loads contiguously as [18,128] (18 descriptors) and is PE-
        # transposed to [128,18] - the direct gather would be 2304 4-byte
        # descriptors hogging the DMA queues.
        bqk_raw = const_pool.tile([18, 128], f32, name="bqk_raw")
        nc.scalar.dma_start(bqk_raw[:], bqkv_d[:].rearrange("(m p) -> m p", m=18))
        bqk = const_pool.tile([128, 18], f32, name="bqk")
        bv_bc = const_pool.tile([128, C], f32, name="bv_bc")
        nc.scalar.dma_start(bv_bc[0:1, :], bqkv_d[2 * C : 3 * C][None, :])
        bo_bc = const_pool.tile([128, C], f32, name="bo_bc")
        nc.scalar.dma_start(bo_bc[0:1, :], bout_d[:][None, :])

        x_tiles = []
        for tb in range(TB):
            x_bf = xin_pool.tile([128, C], bf16, name="x_bf", tag="x_bf", bufs=TB)
            nc.gpsimd.dma_start(x_bf[:], x_d[tb * 128 : (tb + 1) * 128, :])
            x_tiles.append(x_bf)
            if tb == 1:
                wv_dma(0)
            if tb == 3:
                nc.gpsimd.partition_broadcast(bv_bc[:], bv_bc[0:1, :])
            if tb == 5:
                wv_dma(1)

        def issue_wqk(j):
            wqk = wqk_pool.tile([128, CB, 2, 128], bf16, name="wqk", tag="wqk")
            for qk in range(2):
                nc.gpsimd.dma_start(
                    wqk[:, :, qk, :],
                    wqkv_d[:, qk * C + j * 128 : qk * C + (j + 1) * 128].rearrange(
                        "(cb p) f -> p cb f", p=128
                    ),
                )
            return wqk

        wqk0 = issue_wqk(0)

        ones96 = const_pool.tile([128, TB * H], f32, name="ones96")
        nc.gpsimd.memset(ones96[:], 1.0)
        nc.vector.tensor_copy(
            V[:, :, :, D], ones96[:].rearrange("p (t h) -> p t h", t=TB)
        )
        nc.gpsimd.partition_broadcast(bo_bc[:], bo_bc[0:1, :])

        # wout DMA is deferred into the attention phase (the preamble is
        # HBM-bandwidth-bound; wout is not needed until the tail).
        wout = wres.tile([128, CB, C], bf16, name="wout")

        def issue_wout():
            nc.gpsimd.dma_start(
                wout[:, :, :], wout_d[:, :].rearrange("(cb p) f -> p cb f", p=128)
            )

        def proj_group_emitters(j, wqk, qkt):
            # one group per (qk, t-half): 6-mm f32r chain -> bias-add evac
            # casting to bf16 qkt. tch=0 groups only need xT t<512 (tb 0-3).
            ems = []
            for tch in range(2):
                for qk in range(2):
                    def g(qk=qk, tch=tch):
                        ps = st_psum.tile([128, 512], f32, name="ps_qk", tag="st")
                        for cb in range(CB):
                            nc.tensor.matmul(
                                ps[:],
                                wqk[:, cb, qk, :],
                                xT[:, cb, tch * 512 : (tch + 1) * 512],
                                start=(cb == 0),
                                stop=(cb == CB - 1),
                            )
                        m = qk * 6 + j
                        nc.vector.tensor_scalar_add(
                            qkt[:, qk, tch * 512 : (tch + 1) * 512],
                            ps[:],
                            bqk[:, m : m + 1],
                        )
                    ems.append(g)
            return ems

        # ---------- preamble: transpose x, project V, project Q/K pair 0 ----
        qkt = qkt_pool.tile([128, 2, T], bf16, name="qkt", tag="qkt")
        pre_projs = proj_group_emitters(0, wqk0, qkt)

        def v_chain(tb, ch):
            ps = o_psum.tile([128, 512], f32, name="ps_v", tag="ot")
            for cb in range(CB):
                nc.tensor.matmul(
                    ps[:, 0:384],
                    xT[:, cb, tb * 128 : (tb + 1) * 128],
                    wv[:, cb, ch * 384 : (ch + 1) * 384],
                    start=(cb == 0),
                    stop=(cb == CB - 1),
                )
            nc.vector.tensor_add(
                V[:, tb, ch * 6 : (ch + 1) * 6, 0:D],
                ps[:, 0:384].rearrange("p (h d) -> p h d", h=6),
                bv_bc[:, ch * 384 : (ch + 1) * 384].rearrange("p (h d) -> p h d", h=6),
            )

        for tb in range(TB):
            tp = st_psum.tile([128, C], bf16, name="tp", tag="st")
            for c in range(CB):
                nc.tensor.transpose(
                    tp[:, c * 128 : (c + 1) * 128],
                    x_tiles[tb][:, c * 128 : (c + 1) * 128],
                    ident_bf[:],
                )
            eng = nc.scalar if tb % 2 == 0 else nc.vector
            if tb % 2 == 0:
                nc.scalar.copy(
                    xT[:, :, tb * 128 : (tb + 1) * 128],
                    tp[:, 0:C].rearrange("p (c t) -> p c t", c=CB),
                )
            else:
                nc.vector.tensor_copy(
                    xT[:, :, tb * 128 : (tb + 1) * 128],
                    tp[:, 0:C].rearrange("p (c t) -> p c t", c=CB),
                )
            v_chain(tb, 0)
            v_chain(tb, 1)
            if tb == 0:
                bps = st_psum.tile([128, 512], f32, name="bps", tag="st")
                nc.tensor.transpose(bps[:, 0:18], bqk_raw[:], ident[0:18, 0:18])
                nc.vector.tensor_copy(bqk[:], bps[:, 0:18])
            if tb == 3:
                pre_projs[0]()  # (t-half 0, q)
                pre_projs[1]()  # (t-half 0, k)
            if tb == 7:
                pre_projs[2]()
                pre_projs[3]()

        # ---------- attention ----------
        def norm(j, i, ot_qc, qc, variant):
            # OT[j][head-half, qc-chunk] = O'(0:D) * (1 / O'(D)) broadcast
            if variant == "act":
                lns = small_pool.tile([1, 512], f32, name="lns", tag="lns")
                nc.scalar.activation(lns[:], ot_qc[D : D + 1, :], Ln)
                recip = small_pool.tile([1, 512], f32, name="recip", tag="recip")
                nc.scalar.activation(recip[:], lns[:], Exp, scale=-1.0)
            else:
                dn = small_pool.tile([1, 512], f32, name="dn", tag="dn")
                nc.vector.tensor_copy(dn[:], ot_qc[D : D + 1, :])
                recip = small_pool.tile([1, 512], f32, name="recipd", tag="recipd")
                nc.vector.reciprocal_approx_fast(recip[:], dn[:])
            rbc = small_pool.tile([64, 512], f32, name="rbc", tag="rbc")
            nc.gpsimd.partition_broadcast(rbc[:], recip[:])
            nc.vector.tensor_mul(
                OT[j][i * 64 : (i + 1) * 64, qc * 512 : (qc + 1) * 512],
                ot_qc[0:D, :],
                rbc[:],
            )

        deferred = [None]
        for j in range(J):
            if j < J - 1:
                wqk_next = issue_wqk(j + 1)
                qkt_next = qkt_pool.tile([128, 2, T], bf16, name="qkt", tag="qkt")
                pending = proj_group_emitters(j + 1, wqk_next, qkt_next)
            else:
                pending = []
            if j == 2:
                issue_wout()

            # ---- pair-interleaved packed-S attention ----
            # Both heads of the pair advance in lockstep per 512-wide S chunk:
            # S_A and S_B are emitted adjacently so their K=64 matmuls run
            # CONCURRENTLY in separate PE row-groups (lhsT base partitions 0 /
            # 64 -> tile_position rows 0-63 / 64-127).  exp lags the S stream
            # by one chunk; PV lags by two full k-blocks; the last two PVs +
            # qc1 normalizes are deferred into the next pair.
            hA, hB = 2 * j, 2 * j + 1
            sts = {}  # (i, kb, hc) -> (st, a, b)
            pts = {}  # (i, kb) -> pt
            ots = {}  # i -> [ot_qc0, ot_qc1]

            def s_chunk(i, kb, hc, sts=sts, qkt=qkt):
                v0 = kb * 128
                a = max(v0, hc * 512)
                b = (hc + 1) * 512
                st = st_psum.tile([128, 512], f32, name="st", tag="st")
                nc.tensor.matmul(
                    st[:, a - hc * 512 : 512],
                    qkt[i * 64 : (i + 1) * 64, 1, v0 : v0 + 128],
                    qkt[i * 64 : (i + 1) * 64, 0, a:b],
                    start=True,
                    stop=True,
                )
                sts[(i, kb, hc)] = (st, a, b)

            def exp_chunk(i, kb, hc, sts=sts, pts=pts):
                v0 = kb * 128
                st, a, b = sts[(i, kb, hc)]
                diag_hc = 0 if kb < 4 else 1
                if hc == diag_hc:
                    pt = pt_pool.tile([128, T], bf16, name="pt", tag="pt")
                    pts[(i, kb)] = pt
                else:
                    pt = pts[(i, kb)]
                nc.scalar.activation(
                    pt[:, a:b], st[:, a - hc * 512 : b - hc * 512], Exp, scale=0.125
                )
                if hc == diag_hc:
                    nc.gpsimd.affine_select(
                        out=pt[:, v0 : v0 + 128],
                        in_=pt[:, v0 : v0 + 128],
                        compare_op=isge,
                        fill=0.0,
                        base=0,
                        channel_multiplier=-1,
                        pattern=[[1, 128]],
                    )

            def pv(i, pkb, last, j=j, ots=ots, pts=pts):
                pv0 = pkb * 128
                h = 2 * j + i
                ot = ots[i]
                for qc in range(pkb // 4, 2):
                    sq = max(pv0, qc * 512)
                    nc.tensor.matmul(
                        ot[qc][:, sq - qc * 512 : 512],
                        V[:, pkb, h, :],
                        pts[(i, pkb)][:, sq : (qc + 1) * 512],
                        start=(pkb == 0),
                        stop=(pkb == 3 + 4 * qc),
                    )
                if pkb == 3:
                    norm(j, i, ot[0], 0, "dve")
                if last:
                    norm(j, i, ot[1], 1, "dve")

            # chunk stream: kb<4 contributes (kb,0),(kb,1); kb>=4 only (kb,1)
            stream = []
            for kb in range(TB):
                if kb < 4:
                    stream.append((kb, 0))
                stream.append((kb, 1))

            for n, (kb, hc) in enumerate(stream):
                s_chunk(0, kb, hc)
                s_chunk(1, kb, hc)
                if n == 1 and deferred[0] is not None:
                    deferred[0]()
                    deferred[0] = None
                if n >= 1:
                    pkb, phc = stream[n - 1]
                    exp_chunk(0, pkb, phc)
                    exp_chunk(1, pkb, phc)
                first_of_kb = (hc == 0) if kb < 4 else True
                if first_of_kb and kb >= 2:
                    if kb == 2:
                        ots[0] = [
                            o_psum.tile([D + 1, 512], f32, name="otA", tag="ot")
                            for _ in range(2)
                        ]
                        ots[1] = [
                            o_psum.tile([D + 1, 512], f32, name="otB", tag="ot")
                            for _ in range(2)
                        ]
                    pv(0, kb - 2, last=False)
                    pv(1, kb - 2, last=False)
                if n in (2, 5, 8, 10) and pending:
                    pending.pop(0)()

            # pair tail: exp the final chunk now; PV(6), PV(7) + qc1
            # normalizes deferred into the next pair's pipeline.
            exp_chunk(0, 7, 1)
            exp_chunk(1, 7, 1)

            def make_deferred(pv=pv):
                def d():
                    pv(0, 6, last=False)
                    pv(1, 6, last=False)
                    pv(0, 7, last=True)
                    pv(1, 7, last=True)
                return d

            deferred[0] = make_deferred()

            for g in pending:
                g()
            if j < J - 1:
                qkt = qkt_next

        if deferred[0] is not None:
            deferred[0]()
            deferred[0] = None

        # ---------- output projection ----------
        for tb in range(TB):
            yt = y_pool.tile([128, C], f32, name="yt", tag="yt")
            for ch in range(2):
                ps = o_psum.tile([128, 512], f32, name="ps_y", tag="ot")
                for cb in range(CB):
                    nc.tensor.matmul(
                        ps[:, 0:384],
                        OT[cb][:, tb * 128 : (tb + 1) * 128],
                        wout[:, cb, ch * 384 : (ch + 1) * 384],
                        start=(cb == 0),
                        stop=(cb == CB - 1),
                    )
                nc.vector.tensor_add(
                    yt[:, ch * 384 : (ch + 1) * 384],
                    ps[:, 0:384],
                    bo_bc[:, ch * 384 : (ch + 1) * 384],
                )
            nc.sync.dma_start(y_d[tb * 128 : (tb + 1) * 128, :], yt[:])


def build():
    if "nc" in _CACHE:
        return _CACHE["nc"]
    _ensure_path()
    import concourse.bacc as bacc
    import concourse.mybir as mybir
    import concourse.tile as tile
    from concourse.masks import make_identity

    nc = bacc.Bacc(
        "TRN2",
        target_bir_lowering=False,
        debug=False,
        enable_asserts=False,
        num_devices=NCORES,
    )
    with tile.TileContext(nc) as tc:
        _emit(nc, tc, tile, mybir, make_identity)

    # Both Exp and Ln live in the 'natural_log_exp_and_others' ACT table set,
    # but the table-load pass maps Exp to the first set containing it
    # ('exp_and_others'), so Exp/Ln ping-pong table loads every head
    # (~1.3us each).  Restrict Exp membership to the natural_log set for the
    # duration of compile; dict order (= act_func_set_id) is preserved.
    orig_tables = bacc.get_activation_tables

    def _pinned_tables(arch):
        tables = orig_tables(arch)
        exp_t = mybir.ActivationFunctionType.Exp
        if any(exp_t in fns for name, fns in tables.items() if "natural_log" in name):
            for name, fns in tables.items():
                if "natural_log" not in name:
                    fns.discard(exp_t)
        return tables

    bacc.get_activation_tables = _pinned_tables
    try:
        nc.compile()
    finally:
        bacc.get_activation_tables = orig_tables
    _CACHE["nc"] = nc
    return nc


def _in_maps(x, W_qkv, b_qkv, W_out, b_out):
    x = np.ascontiguousarray(np.asarray(x, dtype=np.float32))
    W_qkv = np.ascontiguousarray(np.asarray(W_qkv, dtype=np.float32))
    b_qkv = np.ascontiguousarray(np.asarray(b_qkv, dtype=np.float32))
    W_out = np.ascontiguousarray(np.asarray(W_out, dtype=np.float32))
    b_out = np.ascontiguousarray(np.asarray(b_out, dtype=np.float32))
    return [
        {
            "x": x[b],
            "W_qkv": W_qkv,
            "b_qkv": b_qkv,
            "W_out": W_out,
            "b_out": b_out,
        }
        for b in range(B)
    ]


def _install_ntff_hook():
    """The image's antenv package lacks axon_hooks; synthesize it so
    run_bass_kernel_spmd(trace=True) can NTFF-profile via libaxon_pjrt.so."""
    import sys
    import types

    if "antenv.axon_hooks" in sys.modules:
        return
    mod = types.ModuleType("antenv.axon_hooks")
    state = {"hook": None}
    mod.set_axon_ntff_profile_hook = lambda h: state.__setitem__("hook", h)
    mod.get_axon_ntff_profile_hook = lambda: state["hook"]
    sys.modules["antenv.axon_hooks"] = mod
    import antenv

    antenv.axon_hooks = mod
    try:
        if "/root/.axon_site" not in sys.path:
            sys.path.append("/root/.axon_site")
        from trn_agent_boot.trn_boot import _ntff_profile_via_ctypes

        mod.set_axon_ntff_profile_hook(
            _ntff_profile_via_ctypes("/opt/axon/libaxon_pjrt.so")
        )
    except Exception as exc:  # degrade to no tracing
        print(f"ntff hook unavailable: {exc}", file=sys.stderr)


def run(x, W_qkv, b_qkv, W_out, b_out, trace=False):
    _ensure_path()
    if trace:
        _install_ntff_hook()
    from concourse.bass_utils import run_bass_kernel_spmd

    nc = build()
    res = run_bass_kernel_spmd(
        nc,
        _in_maps(x, W_qkv, b_qkv, W_out, b_out),
        core_ids=list(range(NCORES)),
        trace=trace,
    )
    y = np.stack([res.results[b]["y_out"] for b in range(B)], axis=0)
    return y.astype(np.float32, copy=False), res


def kernel(x, W_qkv, b_qkv, W_out, b_out):
    y, _ = run(x, W_qkv, b_qkv, W_out, b_out, trace=False)
    return y
